# revision 4
# baseline (speedup 1.0000x reference)
"""Trainium2 Bass kernel for the L2D job-shop GNN encoder.

Problem: B=64 batches, J=50 jobs x M=20 machine-ops = N=1000 nodes, E=128,
FF=512, L=3 GNN layers.  Data-parallel over 8 NeuronCores (8 batches each).

Key algebraic restructure vs. the reference:
  - adj_prec aggregation == shift h by one token (zero at job boundaries)
  - adj_mach aggregation == P @ (P^T h) - h  with P the [N,20] one-hot of
    machine ids  ->  two tiny matmuls instead of a [1000x1000] dense matmul.
    (P S) @ Wm == P @ (S @ Wm), and the "- h @ Wm" term folds into
    (Ws - Wm) as the self-weight.
Everything runs feature-major (H = h^T, [E=128 partitions, N tokens free]);
token-contractions (machine segment-sum, final gather) use 8 TensorE
transposes per batch/layer into token-major [125,128] chunks.
"""

import numpy as np

B, J, M = 64, 50, 20
N = J * M            # 1000
E, FF, L = 128, 512, 3
NCORES = 8
BPC = B // NCORES    # 8 batches per core
CH = 8               # token chunks per batch
CP = N // CH         # 125 tokens per chunk
HALF = N // 2        # 500 (psum bank = 512 fp32)

_CACHE = {}


def _build_nc():
    import concourse.bass as bass  # noqa: F401
    import concourse.mybir as mybir
    import concourse.tile as tile
    from concourse import bacc
    from concourse.masks import make_identity

    dt = mybir.dt
    f32 = dt.float32
    i32 = dt.int32
    u8 = dt.uint8
    AF = mybir.ActivationFunctionType
    OP = mybir.AluOpType

    nc = bacc.Bacc(
        "TRN2",
        target_bir_lowering=False,
        debug=False,
        enable_asserts=False,
        num_devices=NCORES,
    )

    proc = nc.dram_tensor("proc_time", [BPC, N], f32, kind="ExternalInput")
    mo = nc.dram_tensor("machine_order", [BPC, N], i32, kind="ExternalInput")
    nxt = nc.dram_tensor("next_op_idx", [BPC, J], i32, kind="ExternalInput")
    fin = nc.dram_tensor("finished_jobs", [BPC, J], u8, kind="ExternalInput")
    w_init = nc.dram_tensor("W_init", [1, E], f32, kind="ExternalInput")
    b_init = nc.dram_tensor("b_init", [E], f32, kind="ExternalInput")
    Ws_d, Wp_d, Wm_d, b_d, W1_d, b1_d, W2_d, b2_d = [], [], [], [], [], [], [], []
    for l in range(L):
        Ws_d.append(nc.dram_tensor(f"Ws{l}", [E, E], f32, kind="ExternalInput"))
        Wp_d.append(nc.dram_tensor(f"Wp{l}", [E, E], f32, kind="ExternalInput"))
        Wm_d.append(nc.dram_tensor(f"Wm{l}", [E, E], f32, kind="ExternalInput"))
        b_d.append(nc.dram_tensor(f"b{l}", [E], f32, kind="ExternalInput"))
        W1_d.append(nc.dram_tensor(f"W1{l}", [E, FF], f32, kind="ExternalInput"))
        b1_d.append(nc.dram_tensor(f"b1{l}", [FF], f32, kind="ExternalInput"))
        W2_d.append(nc.dram_tensor(f"W2{l}", [FF, E], f32, kind="ExternalInput"))
        b2_d.append(nc.dram_tensor(f"b2{l}", [E], f32, kind="ExternalInput"))
    h_out = nc.dram_tensor("h_out", [BPC, N, E], f32, kind="ExternalOutput")
    je_out = nc.dram_tensor("je_out", [BPC, J, E], f32, kind="ExternalOutput")

    with tile.TileContext(nc) as tc:
        with (
            tc.tile_pool(name="const", bufs=1) as const,
            tc.tile_pool(name="wpool", bufs=1) as wpool,
            tc.tile_pool(name="inp", bufs=2) as inp,
            tc.tile_pool(name="hpool", bufs=5) as hpool,
            tc.tile_pool(name="apool", bufs=2) as apool,
            tc.tile_pool(name="htokp", bufs=2) as htokp,
            tc.tile_pool(name="msgp", bufs=4) as msgp,
            tc.tile_pool(name="tpool", bufs=5) as tpool,
            tc.tile_pool(name="smallsb", bufs=3) as smallsb,
            tc.tile_pool(name="tp_ps", bufs=2, space="PSUM") as tp_ps,
            tc.tile_pool(name="small_ps", bufs=2, space="PSUM") as small_ps,
            tc.tile_pool(name="mt_ps", bufs=2, space="PSUM") as mt_ps,
            tc.tile_pool(name="hn_ps", bufs=2, space="PSUM") as hn_ps,
        ):
            # ---------------- constants ----------------
            ident = const.tile([128, 128], f32)
            make_identity(nc, ident[:])
            ones = const.tile([1, 128], f32)
            nc.gpsimd.memset(ones[:], 1.0)
            # iota20f[p,0] = p  (machine id per partition, 20 rows)
            iota20i = const.tile([20, 1], i32)
            nc.gpsimd.iota(iota20i[:], pattern=[[0, 1]], base=0, channel_multiplier=1)
            iota20f = const.tile([20, 1], f32)
            nc.vector.tensor_copy(iota20f[:], iota20i[:])
            # tokidf[p,c] = p + 125*c   (token id of partition p in chunk c)
            tokidi = const.tile([CP, CH], i32)
            nc.gpsimd.iota(tokidi[:], pattern=[[CP, CH]], base=0, channel_multiplier=1)
            tokidf = const.tile([CP, CH], f32)
            nc.vector.tensor_copy(tokidf[:], tokidi[:])
            # iotamf[p, c, m] = m  (machine-id ramp per chunk)
            iotami = const.tile([CP, CH * M], i32)
            nc.gpsimd.iota(
                iotami[:], pattern=[[0, CH], [1, M]], base=0, channel_multiplier=0
            )
            iotamf = const.tile([CP, CH * M], f32)
            nc.vector.tensor_copy(iotamf[:], iotami[:])
            # iotaj20f[0,j] = 20*j
            iotaji = const.tile([1, J], i32)
            nc.gpsimd.iota(iotaji[:], pattern=[[M, J]], base=0, channel_multiplier=0)
            iotajf = const.tile([1, J], f32)
            nc.vector.tensor_copy(iotajf[:], iotaji[:])

            # ---------------- weights ----------------
            winit_sb = wpool.tile([1, E], f32)
            nc.sync.dma_start(winit_sb[:], w_init[:])
            binit_sb = wpool.tile([E, 1], f32)
            nc.sync.dma_start(binit_sb[:], b_init.rearrange("(p o) -> p o", o=1))
            wsm_sb, wp_sb, wm_sb, b_sb, w1_sb, b1_sb, w2_sb, b2_sb = (
                [], [], [], [], [], [], [], []
            )
            for l in range(L):
                ws_t = wpool.tile([E, E], f32, name=f"ws{l}")
                nc.sync.dma_start(ws_t[:], Ws_d[l][:])
                wp_t = wpool.tile([E, E], f32, name=f"wp{l}")
                nc.sync.dma_start(wp_t[:], Wp_d[l][:])
                wm_t = wpool.tile([E, E], f32, name=f"wm{l}")
                nc.sync.dma_start(wm_t[:], Wm_d[l][:])
                wsm_t = wpool.tile([E, E], f32, name=f"wsm{l}")
                nc.vector.tensor_sub(wsm_t[:], ws_t[:], wm_t[:])
                b_t = wpool.tile([E, 1], f32, name=f"b{l}")
                nc.sync.dma_start(b_t[:], b_d[l].rearrange("(p o) -> p o", o=1))
                w1_t = wpool.tile([E, FF], f32, name=f"w1{l}")
                nc.sync.dma_start(w1_t[:], W1_d[l][:])
                b1_t = wpool.tile([E, FF // E], f32, name=f"b1{l}")
                nc.sync.dma_start(b1_t[:], b1_d[l].rearrange("(f p) -> p f", p=E))
                w2_t = wpool.tile([E, FF // E, E], f32, name=f"w2{l}")
                nc.sync.dma_start(w2_t[:], W2_d[l].rearrange("(f p) e -> p f e", p=E))
                b2_t = wpool.tile([E, 1], f32, name=f"b2{l}")
                nc.sync.dma_start(b2_t[:], b2_d[l].rearrange("(p o) -> p o", o=1))
                wsm_sb.append(wsm_t)
                wp_sb.append(wp_t)
                wm_sb.append(wm_t)
                b_sb.append(b_t)
                w1_sb.append(w1_t)
                b1_sb.append(b1_t)
                w2_sb.append(w2_t)
                b2_sb.append(b2_t)

            nf = FF // E  # 4

            for b in range(BPC):
                # ---------- per-batch input prep ----------
                mo_i = inp.tile([1, N], i32, name="mo_i")
                nc.sync.dma_start(mo_i[:], mo[b][None, :])
                mo_f = inp.tile([1, N], f32, name="mo_f")
                nc.vector.tensor_copy(mo_f[:], mo_i[:])
                motok_i = inp.tile([CP, CH], i32, name="motok_i")
                nc.sync.dma_start(motok_i[:], mo[b].rearrange("(c p) -> p c", p=CP))
                motok_f = inp.tile([CP, CH], f32, name="motok_f")
                nc.vector.tensor_copy(motok_f[:], motok_i[:])
                # P_all[p, c, m] = 1.0 if machine_order[token c*125+p] == m
                p_all = inp.tile([CP, CH, M], f32, name="p_all")
                nc.vector.tensor_tensor(
                    p_all[:],
                    motok_f[:][:, :, None].broadcast_to([CP, CH, M]),
                    iotamf[:].rearrange("p (c m) -> p c m", m=M),
                    op=OP.is_equal,
                )
                # PT[m, n] = 1.0 if machine_order[n] == m   [20, 1000]
                pt = inp.tile([20, N], f32, name="pt")
                for hf in range(2):
                    bc_ps = small_ps.tile([20, HALF], f32, name="bc_ps", tag="sp")
                    nc.tensor.matmul(
                        bc_ps[:],
                        ones[:1, :20],
                        mo_f[:1, hf * HALF:(hf + 1) * HALF],
                    )
                    nc.vector.tensor_scalar(
                        pt[:, hf * HALF:(hf + 1) * HALF],
                        bc_ps[:],
                        iota20f[:, 0:1],
                        None,
                        op0=OP.is_equal,
                    )
                # gather one-hots G[p, c, j] = 1.0 if flat[j] == c*125+p
                nxt_i = inp.tile([1, J], i32, name="nxt_i")
                nc.sync.dma_start(nxt_i[:], nxt[b][None, :])
                fin_u = inp.tile([1, J], u8, name="fin_u")
                nc.sync.dma_start(fin_u[:], fin[b][None, :])
                nxt_f = inp.tile([1, J], f32, name="nxt_f")
                nc.vector.tensor_copy(nxt_f[:], nxt_i[:])
                fin_f = inp.tile([1, J], f32, name="fin_f")
                nc.vector.tensor_copy(fin_f[:], fin_u[:])
                flat_f = inp.tile([1, J], f32, name="flat_f")
                # flat = 20*j + next + fin*(19 - next)
                nc.vector.tensor_scalar(
                    flat_f[:], nxt_f[:], -1.0, 19.0, op0=OP.mult, op1=OP.add
                )
                nc.vector.tensor_mul(flat_f[:], flat_f[:], fin_f[:])
                nc.vector.tensor_add(flat_f[:], flat_f[:], nxt_f[:])
                nc.vector.tensor_add(flat_f[:], flat_f[:], iotajf[:])
                g_all = inp.tile([CP, CH, J], f32, name="g_all")
                fb_ps = small_ps.tile([CP, J], f32, name="fb_ps", tag="sp")
                nc.tensor.matmul(fb_ps[:], ones[:1, :CP], flat_f[:1, :])
                for c in range(CH):
                    nc.vector.tensor_scalar(
                        g_all[:, c, :],
                        fb_ps[:],
                        tokidf[:, c:c + 1],
                        None,
                        op0=OP.is_equal,
                    )

                # ---------- init embed ----------
                dur = inp.tile([1, N], f32, name="dur")
                nc.sync.dma_start(dur[:], proc[b][None, :])
                h_cur = hpool.tile([E, N], f32, name="h0", tag="h")
                for hf in range(2):
                    h0_ps = mt_ps.tile([E, HALF], f32, name="h0_ps", tag="mt")
                    nc.tensor.matmul(
                        h0_ps[:], winit_sb[:1, :], dur[:1, hf * HALF:(hf + 1) * HALF]
                    )
                    nc.scalar.activation(
                        h_cur[:, hf * HALF:(hf + 1) * HALF],
                        h0_ps[:],
                        AF.Identity,
                        bias=binit_sb[:, 0:1],
                    )

                # ---------- GNN layers ----------
                for l in range(L):
                    # precedence shift: agg[i] = h[i+1], zero where i%20==19
                    agg = apool.tile([E, N], f32, name="agg")
                    nc.gpsimd.tensor_copy(agg[:, 0:N - 1], h_cur[:, 1:N])
                    nc.gpsimd.memset(
                        agg[:].rearrange("p (j s) -> p j s", s=M)[:, :, M - 1], 0.0
                    )
                    # token-major copies of h (for machine segment-sum)
                    htok = htokp.tile([CP, CH * E], f32, name="htok")
                    for c in range(CH):
                        t_ps = tp_ps.tile([CP, E], f32, name="t_ps", tag="tp")
                        nc.tensor.transpose(
                            t_ps[:], h_cur[:, c * CP:(c + 1) * CP], ident[:]
                        )
                        nc.scalar.copy(htok[:, c * E:(c + 1) * E], t_ps[:])
                    # S^T[e, m] = sum_tok h[tok, e] * P[tok, m]
                    s_ps = small_ps.tile([E, M], f32, name="s_ps", tag="sp")
                    for c in range(CH):
                        nc.tensor.matmul(
                            s_ps[:],
                            htok[:, c * E:(c + 1) * E],
                            p_all[:, c, :],
                            start=(c == 0),
                            stop=(c == CH - 1),
                        )
                    s_sb = smallsb.tile([E, M], f32, name="s_sb")
                    nc.vector.tensor_copy(s_sb[:], s_ps[:])
                    # U = S @ Wm   [20, 128]
                    u_ps = small_ps.tile([M, E], f32, name="u_ps", tag="sp")
                    nc.tensor.matmul(u_ps[:], s_sb[:], wm_sb[l][:])
                    u_sb = smallsb.tile([M, E], f32, name="u_sb")
                    nc.vector.tensor_copy(u_sb[:], u_ps[:])
                    # msg = relu(h (Ws-Wm) + agg Wp + P U + b)
                    msgs = []
                    for hf in range(2):
                        sl = slice(hf * HALF, (hf + 1) * HALF)
                        m_ps = mt_ps.tile([E, HALF], f32, name="m_ps", tag="mt")
                        nc.tensor.matmul(
                            m_ps[:], wsm_sb[l][:], h_cur[:, sl], start=True, stop=False
                        )
                        nc.tensor.matmul(
                            m_ps[:], wp_sb[l][:], agg[:, sl], start=False, stop=False
                        )
                        nc.tensor.matmul(
                            m_ps[:], u_sb[:], pt[:, sl], start=False, stop=True
                        )
                        msg_t = msgp.tile([E, HALF], f32, name="msg_t")
                        nc.scalar.activation(
                            msg_t[:], m_ps[:], AF.Relu, bias=b_sb[l][:, 0:1]
                        )
                        msgs.append(msg_t)
                    # FFN: h_new = msg + relu(msg W1 + b1) W2 + b2
                    hn_pss = [
                        hn_ps.tile([E, HALF], f32, name="hn_ps0", tag="hn"),
                        hn_ps.tile([E, HALF], f32, name="hn_ps1", tag="hn"),
                    ]
                    for f in range(nf):
                        for hf in range(2):
                            tt_ps = mt_ps.tile([E, HALF], f32, name="tt_ps", tag="mt")
                            nc.tensor.matmul(
                                tt_ps[:], w1_sb[l][:, f * E:(f + 1) * E], msgs[hf][:]
                            )
                            t_sb = tpool.tile([E, HALF], f32, name="t_sb")
                            if f % 2 == 0:
                                nc.vector.tensor_scalar(
                                    t_sb[:],
                                    tt_ps[:],
                                    b1_sb[l][:, f:f + 1],
                                    0.0,
                                    op0=OP.add,
                                    op1=OP.max,
                                )
                            else:
                                nc.scalar.activation(
                                    t_sb[:], tt_ps[:], AF.Relu,
                                    bias=b1_sb[l][:, f:f + 1],
                                )
                            nc.tensor.matmul(
                                hn_pss[hf][:],
                                w2_sb[l][:, f, :],
                                t_sb[:],
                                start=(f == 0),
                                stop=(f == nf - 1),
                            )
                    h_nxt = hpool.tile([E, N], f32, name=f"h{l + 1}", tag="h")
                    for hf in range(2):
                        sl = slice(hf * HALF, (hf + 1) * HALF)
                        nc.vector.scalar_tensor_tensor(
                            h_nxt[:, sl],
                            hn_pss[hf][:],
                            b2_sb[l][:, 0:1],
                            msgs[hf][:],
                            op0=OP.add,
                            op1=OP.add,
                        )
                    h_cur = h_nxt

                # ---------- outputs ----------
                htok_o = htokp.tile([CP, CH * E], f32, name="htok_o")
                for c in range(CH):
                    t_ps = tp_ps.tile([CP, E], f32, name="t_ps", tag="tp")
                    nc.tensor.transpose(
                        t_ps[:], h_cur[:, c * CP:(c + 1) * CP], ident[:]
                    )
                    nc.scalar.copy(htok_o[:, c * E:(c + 1) * E], t_ps[:])
                je_ps = small_ps.tile([E, J], f32, name="je_ps", tag="sp")
                for c in range(CH):
                    nc.tensor.matmul(
                        je_ps[:],
                        htok_o[:, c * E:(c + 1) * E],
                        g_all[:, c, :],
                        start=(c == 0),
                        stop=(c == CH - 1),
                    )
                je_sb = smallsb.tile([E, J], f32, name="je_sb")
                nc.vector.tensor_copy(je_sb[:], je_ps[:])
                jet_ps = small_ps.tile([J, E], f32, name="jet_ps", tag="sp")
                nc.tensor.transpose(jet_ps[:], je_sb[:], ident[:])
                jet_sb = smallsb.tile([J, E], f32, name="jet_sb")
                nc.scalar.copy(jet_sb[:], jet_ps[:])
                nc.sync.dma_start(je_out[b], jet_sb[:])
                nc.sync.dma_start(
                    h_out[b].rearrange("(c p) e -> p c e", p=CP),
                    htok_o[:].rearrange("p (c e) -> p c e", e=E),
                )

    nc.compile()
    return nc


def _get_nc():
    if "nc" not in _CACHE:
        _CACHE["nc"] = _build_nc()
    return _CACHE["nc"]


def make_in_maps(proc_time, machine_order, next_op_idx, finished_jobs, params):
    proc_time = np.asarray(proc_time, dtype=np.float32).reshape(B, N)
    machine_order = np.asarray(machine_order, dtype=np.int32).reshape(B, N)
    next_op_idx = np.asarray(next_op_idx, dtype=np.int32).reshape(B, J)
    finished_jobs = np.asarray(finished_jobs).astype(np.uint8).reshape(B, J)
    wmap = {
        "W_init": np.ascontiguousarray(np.asarray(params["W_init"], np.float32)),
        "b_init": np.ascontiguousarray(np.asarray(params["b_init"], np.float32)),
    }
    for l, lp in enumerate(params["layers"]):
        wmap[f"Ws{l}"] = np.ascontiguousarray(np.asarray(lp["Ws"], np.float32))
        wmap[f"Wp{l}"] = np.ascontiguousarray(np.asarray(lp["Wp"], np.float32))
        wmap[f"Wm{l}"] = np.ascontiguousarray(np.asarray(lp["Wm"], np.float32))
        wmap[f"b{l}"] = np.ascontiguousarray(np.asarray(lp["b"], np.float32))
        wmap[f"W1{l}"] = np.ascontiguousarray(np.asarray(lp["W1"], np.float32))
        wmap[f"b1{l}"] = np.ascontiguousarray(np.asarray(lp["b1"], np.float32))
        wmap[f"W2{l}"] = np.ascontiguousarray(np.asarray(lp["W2"], np.float32))
        wmap[f"b2{l}"] = np.ascontiguousarray(np.asarray(lp["b2"], np.float32))
    in_maps = []
    for c in range(NCORES):
        sl = slice(c * BPC, (c + 1) * BPC)
        m = {
            "proc_time": np.ascontiguousarray(proc_time[sl]),
            "machine_order": np.ascontiguousarray(machine_order[sl]),
            "next_op_idx": np.ascontiguousarray(next_op_idx[sl]),
            "finished_jobs": np.ascontiguousarray(finished_jobs[sl]),
        }
        m.update(wmap)
        in_maps.append(m)
    return in_maps


def assemble(results):
    h = np.concatenate([r["h_out"] for r in results], axis=0).reshape(B, N, E)
    je = np.concatenate([r["je_out"] for r in results], axis=0).reshape(B, J, E)
    return je, h


def run_hw(in_maps, trace=False):
    from concourse.bass_utils import run_bass_kernel_spmd

    nc = _get_nc()
    return run_bass_kernel_spmd(
        nc, in_maps, core_ids=list(range(NCORES)), trace=trace
    )


def kernel(proc_time, machine_order, next_op_idx, finished_jobs, params):
    in_maps = make_in_maps(
        proc_time, machine_order, next_op_idx, finished_jobs, params
    )
    res = run_hw(in_maps, trace=False)
    return assemble(res.results)


# revision 8
# speedup vs baseline: 1.6068x; 1.6068x over previous
"""Trainium2 Bass kernel for the L2D job-shop GNN encoder.

Problem: B=64 batches, J=50 jobs x M=20 machine-ops = N=1000 nodes, E=128,
FF=512, L=3 GNN layers.  Data-parallel over 8 NeuronCores (8 batches each).

Key algebraic restructure vs. the reference:
  - adj_prec aggregation == shift h by one token (zero at job boundaries)
  - adj_mach aggregation == P @ (P^T h) - h  with P the [N,20] one-hot of
    machine ids  ->  two tiny matmuls instead of a [1000x1000] dense matmul.
    (P S) @ Wm == P @ (S @ Wm), and the "- h @ Wm" term folds into
    (Ws - Wm) as the self-weight.
Everything runs feature-major (H = h^T, [E=128 partitions, N tokens free]);
token-contractions (machine segment-sum, final gather) use 8 TensorE
transposes per batch/layer into token-major [125,128] chunks.

All large matmuls run in float32r (single-pass replicated fp32, 4x the
fp32 rate for moving dim >= 256); producers of matmul operands write
float32r so the BIR verifier's rounding requirement is met.
"""

import numpy as np

B, J, M = 64, 50, 20
N = J * M            # 1000
E, FF, L = 128, 512, 3
NCORES = 8
BPC = B // NCORES    # 8 batches per core
CH = 8               # token chunks per batch
CP = N // CH         # 125 tokens per chunk
HALF = N // 2        # 500 (psum bank = 512 fp32)

_CACHE = {}


def _build_nc():
    import concourse.bass as bass  # noqa: F401
    import concourse.mybir as mybir
    import concourse.tile as tile
    from concourse import bacc
    from concourse.masks import make_identity

    dt = mybir.dt
    f32 = dt.float32
    f32r = dt.float32r
    i32 = dt.int32
    u8 = dt.uint8
    AF = mybir.ActivationFunctionType
    OP = mybir.AluOpType

    nc = bacc.Bacc(
        "TRN2",
        target_bir_lowering=False,
        debug=False,
        enable_asserts=False,
        num_devices=NCORES,
    )

    proc = nc.dram_tensor("proc_time", [BPC, N], f32, kind="ExternalInput")
    mo = nc.dram_tensor("machine_order", [BPC, N], i32, kind="ExternalInput")
    nxt = nc.dram_tensor("next_op_idx", [BPC, J], i32, kind="ExternalInput")
    fin = nc.dram_tensor("finished_jobs", [BPC, J], u8, kind="ExternalInput")
    w_init = nc.dram_tensor("W_init", [1, E], f32, kind="ExternalInput")
    b_init = nc.dram_tensor("b_init", [E], f32, kind="ExternalInput")
    Ws_d, Wp_d, Wm_d, b_d, W1_d, b1_d, W2_d, b2_d = [], [], [], [], [], [], [], []
    for l in range(L):
        Ws_d.append(nc.dram_tensor(f"Ws{l}", [E, E], f32, kind="ExternalInput"))
        Wp_d.append(nc.dram_tensor(f"Wp{l}", [E, E], f32, kind="ExternalInput"))
        Wm_d.append(nc.dram_tensor(f"Wm{l}", [E, E], f32, kind="ExternalInput"))
        b_d.append(nc.dram_tensor(f"b{l}", [E], f32, kind="ExternalInput"))
        W1_d.append(nc.dram_tensor(f"W1{l}", [E, FF], f32, kind="ExternalInput"))
        b1_d.append(nc.dram_tensor(f"b1{l}", [FF], f32, kind="ExternalInput"))
        W2_d.append(nc.dram_tensor(f"W2{l}", [FF, E], f32, kind="ExternalInput"))
        b2_d.append(nc.dram_tensor(f"b2{l}", [E], f32, kind="ExternalInput"))
    h_out = nc.dram_tensor("h_out", [BPC, N, E], f32, kind="ExternalOutput")
    je_out = nc.dram_tensor("je_out", [BPC, J, E], f32, kind="ExternalOutput")

    with tile.TileContext(nc) as tc:
        with (
            tc.tile_pool(name="const", bufs=1) as const,
            tc.tile_pool(name="wpool", bufs=1) as wpool,
            tc.tile_pool(name="inp", bufs=2) as inp,
            tc.tile_pool(name="hpool", bufs=5) as hpool,
            tc.tile_pool(name="apool", bufs=2) as apool,
            tc.tile_pool(name="htokp", bufs=2) as htokp,
            tc.tile_pool(name="msgp", bufs=4) as msgp,
            tc.tile_pool(name="tpool", bufs=5) as tpool,
            tc.tile_pool(name="smallsb", bufs=3) as smallsb,
            tc.tile_pool(name="tp_ps", bufs=2, space="PSUM") as tp_ps,
            tc.tile_pool(name="small_ps", bufs=2, space="PSUM") as small_ps,
            tc.tile_pool(name="mt_ps", bufs=2, space="PSUM") as mt_ps,
            tc.tile_pool(name="hn_ps", bufs=2, space="PSUM") as hn_ps,
        ):
            # ---------------- constants ----------------
            ident = const.tile([128, 128], f32)
            make_identity(nc, ident[:])
            ones = const.tile([1, 128], f32)
            nc.gpsimd.memset(ones[:], 1.0)
            iota20i = const.tile([20, 1], i32)
            nc.gpsimd.iota(iota20i[:], pattern=[[0, 1]], base=0, channel_multiplier=1)
            iota20f = const.tile([20, 1], f32)
            nc.vector.tensor_copy(iota20f[:], iota20i[:])
            tokidi = const.tile([CP, CH], i32)
            nc.gpsimd.iota(tokidi[:], pattern=[[CP, CH]], base=0, channel_multiplier=1)
            tokidf = const.tile([CP, CH], f32)
            nc.vector.tensor_copy(tokidf[:], tokidi[:])
            iotami = const.tile([CP, CH * M], i32)
            nc.gpsimd.iota(
                iotami[:], pattern=[[0, CH], [1, M]], base=0, channel_multiplier=0
            )
            iotamf = const.tile([CP, CH * M], f32)
            nc.vector.tensor_copy(iotamf[:], iotami[:])
            iotaji = const.tile([1, J], i32)
            nc.gpsimd.iota(iotaji[:], pattern=[[M, J]], base=0, channel_multiplier=0)
            iotajf = const.tile([1, J], f32)
            nc.vector.tensor_copy(iotajf[:], iotaji[:])
            zero_f = const.tile([E, 1], f32)
            nc.gpsimd.memset(zero_f[:], 0.0)
            zero_r = const.tile([E, 1], f32r)
            nc.vector.tensor_copy(zero_r[:], zero_f[:])

            # ---------------- weights ----------------
            winit_sb = wpool.tile([1, E], f32)
            nc.sync.dma_start(winit_sb[:], w_init[:])
            binit_sb = wpool.tile([E, 1], f32)
            nc.sync.dma_start(binit_sb[:], b_init.rearrange("(p o) -> p o", o=1))
            wsm_sb, wp_sb, wm_sb, b_sb, w1_sb, b1_sb, w2_sb, b2_sb = (
                [], [], [], [], [], [], [], []
            )
            for l in range(L):
                ws_t = wpool.tile([E, E], f32, name=f"ws{l}")
                nc.sync.dma_start(ws_t[:], Ws_d[l][:])
                wpf_t = wpool.tile([E, E], f32, name=f"wpf{l}")
                nc.sync.dma_start(wpf_t[:], Wp_d[l][:])
                wmf_t = wpool.tile([E, E], f32, name=f"wmf{l}")
                nc.sync.dma_start(wmf_t[:], Wm_d[l][:])
                # f32r (rounded) weight copies for the PE
                wsm_t = wpool.tile([E, E], f32r, name=f"wsm{l}")
                nc.vector.tensor_sub(wsm_t[:], ws_t[:], wmf_t[:])
                wp_t = wpool.tile([E, E], f32r, name=f"wp{l}")
                nc.vector.tensor_copy(wp_t[:], wpf_t[:])
                wm_t = wpool.tile([E, E], f32r, name=f"wm{l}")
                nc.vector.tensor_copy(wm_t[:], wmf_t[:])
                b_t = wpool.tile([E, 1], f32, name=f"b{l}")
                nc.sync.dma_start(b_t[:], b_d[l].rearrange("(p o) -> p o", o=1))
                w1f_t = wpool.tile([E, FF], f32, name=f"w1f{l}")
                nc.sync.dma_start(w1f_t[:], W1_d[l][:])
                w1_t = wpool.tile([E, FF], f32r, name=f"w1{l}")
                nc.vector.tensor_copy(w1_t[:], w1f_t[:])
                b1_t = wpool.tile([E, FF // E], f32, name=f"b1{l}")
                nc.sync.dma_start(b1_t[:], b1_d[l].rearrange("(f p) -> p f", p=E))
                w2f_t = wpool.tile([E, FF // E, E], f32, name=f"w2f{l}")
                nc.sync.dma_start(w2f_t[:], W2_d[l].rearrange("(f p) e -> p f e", p=E))
                w2_t = wpool.tile([E, FF // E, E], f32r, name=f"w2{l}")
                nc.vector.tensor_copy(w2_t[:], w2f_t[:])
                b2_t = wpool.tile([E, 1], f32, name=f"b2{l}")
                nc.sync.dma_start(b2_t[:], b2_d[l].rearrange("(p o) -> p o", o=1))
                wsm_sb.append(wsm_t)
                wp_sb.append(wp_t)
                wm_sb.append(wm_t)
                b_sb.append(b_t)
                w1_sb.append(w1_t)
                b1_sb.append(b1_t)
                w2_sb.append(w2_t)
                b2_sb.append(b2_t)

            nf = FF // E  # 4

            for b in range(BPC):
                # ---------- per-batch input prep ----------
                mo_i = inp.tile([1, N], i32, name="mo_i")
                nc.sync.dma_start(mo_i[:], mo[b][None, :])
                mo_f = inp.tile([1, N], f32, name="mo_f")
                nc.vector.tensor_copy(mo_f[:], mo_i[:])
                motok_i = inp.tile([CP, CH], i32, name="motok_i")
                nc.sync.dma_start(motok_i[:], mo[b].rearrange("(c p) -> p c", p=CP))
                motok_f = inp.tile([CP, CH], f32, name="motok_f")
                nc.vector.tensor_copy(motok_f[:], motok_i[:])
                # P_all[p, c, m] = 1.0 if machine_order[token c*125+p] == m
                p_all = inp.tile([CP, CH, M], f32r, name="p_all")
                nc.vector.tensor_tensor(
                    p_all[:],
                    motok_f[:][:, :, None].broadcast_to([CP, CH, M]),
                    iotamf[:].rearrange("p (c m) -> p c m", m=M),
                    op=OP.is_equal,
                )
                # PT[m, n] = 1.0 if machine_order[n] == m   [20, 1000]
                pt = inp.tile([20, N], f32r, name="pt")
                for hf in range(2):
                    bc_ps = small_ps.tile([20, HALF], f32, name="bc_ps", tag="sp")
                    nc.tensor.matmul(
                        bc_ps[:],
                        ones[:1, :20],
                        mo_f[:1, hf * HALF:(hf + 1) * HALF],
                    )
                    nc.vector.tensor_scalar(
                        pt[:, hf * HALF:(hf + 1) * HALF],
                        bc_ps[:],
                        iota20f[:, 0:1],
                        None,
                        op0=OP.is_equal,
                    )
                # gather one-hots G[p, c, j] = 1.0 if flat[j] == c*125+p
                nxt_i = inp.tile([1, J], i32, name="nxt_i")
                nc.sync.dma_start(nxt_i[:], nxt[b][None, :])
                fin_u = inp.tile([1, J], u8, name="fin_u")
                nc.sync.dma_start(fin_u[:], fin[b][None, :])
                nxt_f = inp.tile([1, J], f32, name="nxt_f")
                nc.vector.tensor_copy(nxt_f[:], nxt_i[:])
                fin_f = inp.tile([1, J], f32, name="fin_f")
                nc.vector.tensor_copy(fin_f[:], fin_u[:])
                flat_f = inp.tile([1, J], f32, name="flat_f")
                # flat = 20*j + next + fin*(19 - next)
                nc.vector.tensor_scalar(
                    flat_f[:], nxt_f[:], -1.0, 19.0, op0=OP.mult, op1=OP.add
                )
                nc.vector.tensor_mul(flat_f[:], flat_f[:], fin_f[:])
                nc.vector.tensor_add(flat_f[:], flat_f[:], nxt_f[:])
                nc.vector.tensor_add(flat_f[:], flat_f[:], iotajf[:])
                g_all = inp.tile([CP, CH, J], f32r, name="g_all")
                fb_ps = small_ps.tile([CP, J], f32, name="fb_ps", tag="sp")
                nc.tensor.matmul(fb_ps[:], ones[:1, :CP], flat_f[:1, :])
                for c in range(CH):
                    nc.vector.tensor_scalar(
                        g_all[:, c, :],
                        fb_ps[:],
                        tokidf[:, c:c + 1],
                        None,
                        op0=OP.is_equal,
                    )

                # ---------- init embed ----------
                dur = inp.tile([1, N], f32, name="dur")
                nc.sync.dma_start(dur[:], proc[b][None, :])
                h_cur = hpool.tile([E, N], f32r, name="h0", tag="h")
                for hf in range(2):
                    h0_ps = mt_ps.tile([E, HALF], f32, name="h0_ps", tag="mt")
                    nc.tensor.matmul(
                        h0_ps[:], winit_sb[:1, :], dur[:1, hf * HALF:(hf + 1) * HALF]
                    )
                    nc.scalar.activation(
                        h_cur[:, hf * HALF:(hf + 1) * HALF],
                        h0_ps[:],
                        AF.Identity,
                        bias=binit_sb[:, 0:1],
                    )

                # ---------- GNN layers ----------
                for l in range(L):
                    # precedence shift: agg[i] = h[i+1], zero where i%20==19
                    agg = apool.tile([E, N], f32r, name="agg")
                    agg3 = agg[:].rearrange("p (j s) -> p j s", s=M)
                    h3 = h_cur[:].rearrange("p (j s) -> p j s", s=M)
                    nc.vector.tensor_copy(agg3[:, :, 0:M - 1], h3[:, :, 1:M])
                    nc.vector.tensor_copy(
                        agg3[:, :, M - 1], zero_r[:, 0:1].broadcast_to([E, J])
                    )
                    # token-major copies of h (for machine segment-sum)
                    htok = htokp.tile([CP, CH * E], f32r, name="htok")
                    for c in range(CH):
                        t_ps = tp_ps.tile([CP, E], f32, name="t_ps", tag="tp")
                        nc.tensor.transpose(
                            t_ps[:],
                            h_cur[:, c * CP:(c + 1) * CP].bitcast(f32),
                            ident[:],
                        )
                        nc.scalar.copy(htok[:, c * E:(c + 1) * E], t_ps[:])
                    # S^T[e, m] = sum_tok h[tok, e] * P[tok, m]
                    s_ps = small_ps.tile([E, M], f32, name="s_ps", tag="sp")
                    for c in range(CH):
                        nc.tensor.matmul(
                            s_ps[:],
                            htok[:, c * E:(c + 1) * E],
                            p_all[:, c, :],
                            start=(c == 0),
                            stop=(c == CH - 1),
                        )
                    s_sb = smallsb.tile([E, M], f32r, name="s_sb")
                    nc.vector.tensor_copy(s_sb[:], s_ps[:])
                    # U = S @ Wm   [20, 128]
                    u_ps = small_ps.tile([M, E], f32, name="u_ps", tag="sp")
                    nc.tensor.matmul(u_ps[:], s_sb[:], wm_sb[l][:])
                    u_sb = smallsb.tile([M, E], f32r, name="u_sb")
                    nc.vector.tensor_copy(u_sb[:], u_ps[:])
                    # msg = relu(h (Ws-Wm) + agg Wp + P U + b)
                    msgs = []
                    for hf in range(2):
                        sl = slice(hf * HALF, (hf + 1) * HALF)
                        m_ps = mt_ps.tile([E, HALF], f32, name="m_ps", tag="mt")
                        nc.tensor.matmul(
                            m_ps[:], wsm_sb[l][:], h_cur[:, sl],
                            start=True, stop=False,
                        )
                        nc.tensor.matmul(
                            m_ps[:], wp_sb[l][:], agg[:, sl],
                            start=False, stop=False,
                        )
                        nc.tensor.matmul(
                            m_ps[:], u_sb[:], pt[:, sl], start=False, stop=True
                        )
                        msg_t = msgp.tile([E, HALF], f32r, name="msg_t")
                        nc.scalar.activation(
                            msg_t[:], m_ps[:], AF.Relu, bias=b_sb[l][:, 0:1]
                        )
                        msgs.append(msg_t)
                    # FFN: h_new = msg + relu(msg W1 + b1) W2 + b2
                    hn_pss = [
                        hn_ps.tile([E, HALF], f32, name="hn_ps0", tag="hn"),
                        hn_ps.tile([E, HALF], f32, name="hn_ps1", tag="hn"),
                    ]
                    for f in range(nf):
                        for hf in range(2):
                            tt_ps = mt_ps.tile([E, HALF], f32, name="tt_ps", tag="mt")
                            nc.tensor.matmul(
                                tt_ps[:], w1_sb[l][:, f * E:(f + 1) * E], msgs[hf][:]
                            )
                            t_sb = tpool.tile([E, HALF], f32r, name="t_sb")
                            if f % 2 == 0:
                                nc.vector.tensor_scalar(
                                    t_sb[:],
                                    tt_ps[:],
                                    b1_sb[l][:, f:f + 1],
                                    0.0,
                                    op0=OP.add,
                                    op1=OP.max,
                                )
                            else:
                                nc.scalar.activation(
                                    t_sb[:], tt_ps[:], AF.Relu,
                                    bias=b1_sb[l][:, f:f + 1],
                                )
                            nc.tensor.matmul(
                                hn_pss[hf][:],
                                w2_sb[l][:, f, :],
                                t_sb[:],
                                start=(f == 0),
                                stop=(f == nf - 1),
                            )
                    h_nxt = hpool.tile([E, N], f32r, name=f"h{l + 1}", tag="h")
                    for hf in range(2):
                        sl = slice(hf * HALF, (hf + 1) * HALF)
                        nc.vector.scalar_tensor_tensor(
                            h_nxt[:, sl],
                            hn_pss[hf][:],
                            b2_sb[l][:, 0:1],
                            msgs[hf][:],
                            op0=OP.add,
                            op1=OP.add,
                        )
                    h_cur = h_nxt

                # ---------- outputs ----------
                htok_o = htokp.tile([CP, CH * E], f32r, name="htok_o")
                for c in range(CH):
                    t_ps = tp_ps.tile([CP, E], f32, name="t_ps", tag="tp")
                    nc.tensor.transpose(
                        t_ps[:],
                        h_cur[:, c * CP:(c + 1) * CP].bitcast(f32),
                        ident[:],
                    )
                    nc.scalar.copy(htok_o[:, c * E:(c + 1) * E], t_ps[:])
                je_ps = small_ps.tile([E, J], f32, name="je_ps", tag="sp")
                for c in range(CH):
                    nc.tensor.matmul(
                        je_ps[:],
                        htok_o[:, c * E:(c + 1) * E],
                        g_all[:, c, :],
                        start=(c == 0),
                        stop=(c == CH - 1),
                    )
                je_sb = smallsb.tile([E, J], f32, name="je_sb")
                nc.vector.tensor_copy(je_sb[:], je_ps[:])
                jet_ps = small_ps.tile([J, E], f32, name="jet_ps", tag="sp")
                nc.tensor.transpose(jet_ps[:], je_sb[:], ident[:])
                jet_sb = smallsb.tile([J, E], f32, name="jet_sb")
                nc.scalar.copy(jet_sb[:], jet_ps[:])
                nc.sync.dma_start(je_out[b], jet_sb[:])
                nc.sync.dma_start(
                    h_out[b].rearrange("(c p) e -> p c e", p=CP),
                    htok_o[:].bitcast(f32).rearrange("p (c e) -> p c e", e=E),
                )

    nc.compile()
    return nc


def _get_nc():
    if "nc" not in _CACHE:
        _CACHE["nc"] = _build_nc()
    return _CACHE["nc"]


def make_in_maps(proc_time, machine_order, next_op_idx, finished_jobs, params):
    proc_time = np.asarray(proc_time, dtype=np.float32).reshape(B, N)
    machine_order = np.asarray(machine_order, dtype=np.int32).reshape(B, N)
    next_op_idx = np.asarray(next_op_idx, dtype=np.int32).reshape(B, J)
    finished_jobs = np.asarray(finished_jobs).astype(np.uint8).reshape(B, J)
    wmap = {
        "W_init": np.ascontiguousarray(np.asarray(params["W_init"], np.float32)),
        "b_init": np.ascontiguousarray(np.asarray(params["b_init"], np.float32)),
    }
    for l, lp in enumerate(params["layers"]):
        wmap[f"Ws{l}"] = np.ascontiguousarray(np.asarray(lp["Ws"], np.float32))
        wmap[f"Wp{l}"] = np.ascontiguousarray(np.asarray(lp["Wp"], np.float32))
        wmap[f"Wm{l}"] = np.ascontiguousarray(np.asarray(lp["Wm"], np.float32))
        wmap[f"b{l}"] = np.ascontiguousarray(np.asarray(lp["b"], np.float32))
        wmap[f"W1{l}"] = np.ascontiguousarray(np.asarray(lp["W1"], np.float32))
        wmap[f"b1{l}"] = np.ascontiguousarray(np.asarray(lp["b1"], np.float32))
        wmap[f"W2{l}"] = np.ascontiguousarray(np.asarray(lp["W2"], np.float32))
        wmap[f"b2{l}"] = np.ascontiguousarray(np.asarray(lp["b2"], np.float32))
    in_maps = []
    for c in range(NCORES):
        sl = slice(c * BPC, (c + 1) * BPC)
        m = {
            "proc_time": np.ascontiguousarray(proc_time[sl]),
            "machine_order": np.ascontiguousarray(machine_order[sl]),
            "next_op_idx": np.ascontiguousarray(next_op_idx[sl]),
            "finished_jobs": np.ascontiguousarray(finished_jobs[sl]),
        }
        m.update(wmap)
        in_maps.append(m)
    return in_maps


def assemble(results):
    h = np.concatenate([r["h_out"] for r in results], axis=0).reshape(B, N, E)
    je = np.concatenate([r["je_out"] for r in results], axis=0).reshape(B, J, E)
    return je, h


def run_hw(in_maps, trace=False):
    from concourse.bass_utils import run_bass_kernel_spmd

    nc = _get_nc()
    return run_bass_kernel_spmd(
        nc, in_maps, core_ids=list(range(NCORES)), trace=trace
    )


def kernel(proc_time, machine_order, next_op_idx, finished_jobs, params):
    in_maps = make_in_maps(
        proc_time, machine_order, next_op_idx, finished_jobs, params
    )
    res = run_hw(in_maps, trace=False)
    return assemble(res.results)


# revision 10
# speedup vs baseline: 1.8621x; 1.1589x over previous
"""Trainium2 Bass kernel for the L2D job-shop GNN encoder.

Problem: B=64 batches, J=50 jobs x M=20 machine-ops = N=1000 nodes, E=128,
FF=512, L=3 GNN layers.  Data-parallel over 8 NeuronCores (8 batches each).

Key algebraic restructure vs. the reference:
  - adj_prec aggregation == shift h by one token (zero at job boundaries)
  - adj_mach aggregation == P @ (P^T h) - h  with P the [N,20] one-hot of
    machine ids  ->  two tiny matmuls instead of a [1000x1000] dense matmul.
    (P S) @ Wm == P @ (S @ Wm), and the "- h @ Wm" term folds into
    (Ws - Wm) as the self-weight.
Everything runs feature-major (H = h^T, [E=128 partitions, N tokens free]);
token-contractions (machine segment-sum, final gather) use 8 TensorE
transposes per batch/layer into token-major [125,128] chunks.

All large matmuls run in float32r (single-pass replicated fp32, 4x the
fp32 rate for moving dim >= 256); producers of matmul operands write
float32r so the BIR verifier's rounding requirement is met.
"""

import numpy as np

B, J, M = 64, 50, 20
N = J * M            # 1000
E, FF, L = 128, 512, 3
NCORES = 8
BPC = B // NCORES    # 8 batches per core
CH = 8               # token chunks per batch
CP = N // CH         # 125 tokens per chunk
HALF = N // 2        # 500 (psum bank = 512 fp32)

_CACHE = {}


def _build_nc():
    import concourse.bass as bass  # noqa: F401
    import concourse.mybir as mybir
    import concourse.tile as tile
    from concourse import bacc
    from concourse.masks import make_identity

    dt = mybir.dt
    f32 = dt.float32
    f32r = dt.float32r
    i32 = dt.int32
    u8 = dt.uint8
    AF = mybir.ActivationFunctionType
    OP = mybir.AluOpType

    nc = bacc.Bacc(
        "TRN2",
        target_bir_lowering=False,
        debug=False,
        enable_asserts=False,
        num_devices=NCORES,
    )

    proc = nc.dram_tensor("proc_time", [BPC, N], f32, kind="ExternalInput")
    mo = nc.dram_tensor("machine_order", [BPC, N], i32, kind="ExternalInput")
    nxt = nc.dram_tensor("next_op_idx", [BPC, J], i32, kind="ExternalInput")
    fin = nc.dram_tensor("finished_jobs", [BPC, J], u8, kind="ExternalInput")
    w_init = nc.dram_tensor("W_init", [1, E], f32, kind="ExternalInput")
    b_init = nc.dram_tensor("b_init", [E], f32, kind="ExternalInput")
    Ws_d, Wp_d, Wm_d, b_d, W1_d, b1_d, W2_d, b2_d = [], [], [], [], [], [], [], []
    for l in range(L):
        Ws_d.append(nc.dram_tensor(f"Ws{l}", [E, E], f32, kind="ExternalInput"))
        Wp_d.append(nc.dram_tensor(f"Wp{l}", [E, E], f32, kind="ExternalInput"))
        Wm_d.append(nc.dram_tensor(f"Wm{l}", [E, E], f32, kind="ExternalInput"))
        b_d.append(nc.dram_tensor(f"b{l}", [E], f32, kind="ExternalInput"))
        W1_d.append(nc.dram_tensor(f"W1{l}", [E, FF], f32, kind="ExternalInput"))
        b1_d.append(nc.dram_tensor(f"b1{l}", [FF], f32, kind="ExternalInput"))
        W2_d.append(nc.dram_tensor(f"W2{l}", [FF, E], f32, kind="ExternalInput"))
        b2_d.append(nc.dram_tensor(f"b2{l}", [E], f32, kind="ExternalInput"))
    h_out = nc.dram_tensor("h_out", [BPC, N, E], f32, kind="ExternalOutput")
    je_out = nc.dram_tensor("je_out", [BPC, J, E], f32, kind="ExternalOutput")

    with tile.TileContext(nc) as tc:
        with (
            tc.tile_pool(name="const", bufs=1) as const,
            tc.tile_pool(name="wpool", bufs=1) as wpool,
            tc.tile_pool(name="inp", bufs=2) as inp,
            tc.tile_pool(name="hpool", bufs=5) as hpool,
            tc.tile_pool(name="apool", bufs=2) as apool,
            tc.tile_pool(name="htokp", bufs=3) as htokp,
            tc.tile_pool(name="msgp", bufs=4) as msgp,
            tc.tile_pool(name="tpool", bufs=5) as tpool,
            tc.tile_pool(name="smallsb", bufs=3) as smallsb,
            tc.tile_pool(name="tp_ps", bufs=2, space="PSUM") as tp_ps,
            tc.tile_pool(name="small_ps", bufs=1, space="PSUM") as small_ps,
            tc.tile_pool(name="mt_ps", bufs=3, space="PSUM") as mt_ps,
            tc.tile_pool(name="hn_ps", bufs=2, space="PSUM") as hn_ps,
        ):
            # ---------------- constants ----------------
            ident = const.tile([128, 128], f32)
            make_identity(nc, ident[:])
            ones = const.tile([1, 128], f32)
            nc.gpsimd.memset(ones[:], 1.0)
            iota20i = const.tile([20, 1], i32)
            nc.gpsimd.iota(iota20i[:], pattern=[[0, 1]], base=0, channel_multiplier=1)
            iota20f = const.tile([20, 1], f32)
            nc.vector.tensor_copy(iota20f[:], iota20i[:])
            tokidi = const.tile([CP, CH], i32)
            nc.gpsimd.iota(tokidi[:], pattern=[[CP, CH]], base=0, channel_multiplier=1)
            tokidf = const.tile([CP, CH], f32)
            nc.vector.tensor_copy(tokidf[:], tokidi[:])
            iotami = const.tile([CP, CH * M], i32)
            nc.gpsimd.iota(
                iotami[:], pattern=[[0, CH], [1, M]], base=0, channel_multiplier=0
            )
            iotamf = const.tile([CP, CH * M], f32)
            nc.vector.tensor_copy(iotamf[:], iotami[:])
            iotaji = const.tile([1, J], i32)
            nc.gpsimd.iota(iotaji[:], pattern=[[M, J]], base=0, channel_multiplier=0)
            iotajf = const.tile([1, J], f32)
            nc.vector.tensor_copy(iotajf[:], iotaji[:])
            zero_f = const.tile([E, 1], f32)
            nc.gpsimd.memset(zero_f[:], 0.0)
            zero_r = const.tile([E, 1], f32r)
            nc.vector.tensor_copy(zero_r[:], zero_f[:])
            ident_r = const.tile([128, 128], f32r)
            nc.vector.tensor_copy(ident_r[:], ident[:])

            # ---------------- weights ----------------
            winit_sb = wpool.tile([E, 1], f32)
            nc.sync.dma_start(winit_sb[:], w_init.rearrange("o p -> p o"))
            binit_sb = wpool.tile([E, 1], f32)
            nc.sync.dma_start(binit_sb[:], b_init.rearrange("(p o) -> p o", o=1))
            wsm_sb, wp_sb, wm_sb, b_sb, w1_sb, b1_sb, w2_sb, b2_sb = (
                [], [], [], [], [], [], [], []
            )
            for l in range(L):
                ws_t = wpool.tile([E, E], f32, name=f"ws{l}")
                nc.sync.dma_start(ws_t[:], Ws_d[l][:])
                wpf_t = wpool.tile([E, E], f32, name=f"wpf{l}")
                nc.sync.dma_start(wpf_t[:], Wp_d[l][:])
                wmf_t = wpool.tile([E, E], f32, name=f"wmf{l}")
                nc.sync.dma_start(wmf_t[:], Wm_d[l][:])
                # f32r (rounded) weight copies for the PE
                wsm_t = wpool.tile([E, E], f32r, name=f"wsm{l}")
                nc.vector.tensor_sub(wsm_t[:], ws_t[:], wmf_t[:])
                wp_t = wpool.tile([E, E], f32r, name=f"wp{l}")
                nc.vector.tensor_copy(wp_t[:], wpf_t[:])
                wm_t = wpool.tile([E, E], f32r, name=f"wm{l}")
                nc.vector.tensor_copy(wm_t[:], wmf_t[:])
                b_t = wpool.tile([E, 1], f32, name=f"b{l}")
                nc.sync.dma_start(b_t[:], b_d[l].rearrange("(p o) -> p o", o=1))
                w1f_t = wpool.tile([E, FF], f32, name=f"w1f{l}")
                nc.sync.dma_start(w1f_t[:], W1_d[l][:])
                w1_t = wpool.tile([E, FF], f32r, name=f"w1{l}")
                nc.vector.tensor_copy(w1_t[:], w1f_t[:])
                b1_t = wpool.tile([E, FF // E], f32, name=f"b1{l}")
                nc.sync.dma_start(b1_t[:], b1_d[l].rearrange("(f p) -> p f", p=E))
                w2f_t = wpool.tile([E, FF // E, E], f32, name=f"w2f{l}")
                nc.sync.dma_start(w2f_t[:], W2_d[l].rearrange("(f p) e -> p f e", p=E))
                w2_t = wpool.tile([E, FF // E, E], f32r, name=f"w2{l}")
                nc.vector.tensor_copy(w2_t[:], w2f_t[:])
                b2_t = wpool.tile([E, 1], f32, name=f"b2{l}")
                nc.sync.dma_start(b2_t[:], b2_d[l].rearrange("(p o) -> p o", o=1))
                wsm_sb.append(wsm_t)
                wp_sb.append(wp_t)
                wm_sb.append(wm_t)
                b_sb.append(b_t)
                w1_sb.append(w1_t)
                b1_sb.append(b1_t)
                w2_sb.append(w2_t)
                b2_sb.append(b2_t)

            nf = FF // E  # 4

            for b in range(BPC):
                # ---------- per-batch input prep ----------
                mo_bc = inp.tile([20, N], i32, name="mo_bc")
                nc.sync.dma_start(mo_bc[:], mo[b].partition_broadcast(20))
                motok_i = inp.tile([CP, CH], i32, name="motok_i")
                nc.sync.dma_start(motok_i[:], mo[b].rearrange("(c p) -> p c", p=CP))
                motok_f = inp.tile([CP, CH], f32, name="motok_f")
                nc.vector.tensor_copy(motok_f[:], motok_i[:])
                # P_all[p, c, m] = 1.0 if machine_order[token c*125+p] == m
                p_all = inp.tile([CP, CH, M], f32r, name="p_all")
                nc.vector.tensor_tensor(
                    p_all[:],
                    motok_f[:][:, :, None].broadcast_to([CP, CH, M]),
                    iotamf[:].rearrange("p (c m) -> p c m", m=M),
                    op=OP.is_equal,
                )
                # PT[m, n] = 1.0 if machine_order[n] == m   [20, 1000]
                pt = inp.tile([20, N], f32r, name="pt")
                nc.vector.tensor_scalar(
                    pt[:],
                    mo_bc[:],
                    iota20f[:, 0:1],
                    None,
                    op0=OP.is_equal,
                )
                # gather one-hots G[p, c, j] = 1.0 if flat[j] == c*125+p
                nxt_i = inp.tile([1, J], i32, name="nxt_i")
                nc.sync.dma_start(nxt_i[:], nxt[b][None, :])
                fin_u = inp.tile([1, J], u8, name="fin_u")
                nc.sync.dma_start(fin_u[:], fin[b][None, :])
                nxt_f = inp.tile([1, J], f32, name="nxt_f")
                nc.vector.tensor_copy(nxt_f[:], nxt_i[:])
                fin_f = inp.tile([1, J], f32, name="fin_f")
                nc.vector.tensor_copy(fin_f[:], fin_u[:])
                flat_f = inp.tile([1, J], f32, name="flat_f")
                # flat = 20*j + next + fin*(19 - next)
                nc.vector.tensor_scalar(
                    flat_f[:], nxt_f[:], -1.0, 19.0, op0=OP.mult, op1=OP.add
                )
                nc.vector.tensor_mul(flat_f[:], flat_f[:], fin_f[:])
                nc.vector.tensor_add(flat_f[:], flat_f[:], nxt_f[:])
                nc.vector.tensor_add(flat_f[:], flat_f[:], iotajf[:])
                g_all = inp.tile([CP, CH, J], f32r, name="g_all")
                fb_ps = small_ps.tile([CP, J], f32, name="fb_ps", tag="sp")
                nc.tensor.matmul(fb_ps[:], ones[:1, :CP], flat_f[:1, :])
                for c in range(CH):
                    nc.vector.tensor_scalar(
                        g_all[:, c, :],
                        fb_ps[:],
                        tokidf[:, c:c + 1],
                        None,
                        op0=OP.is_equal,
                    )

                # ---------- init embed: h0 = dur * W_init + b_init ----------
                dur_bc = inp.tile([E, N], f32, name="dur_bc")
                nc.sync.dma_start(dur_bc[:], proc[b].partition_broadcast(E))
                h_cur = hpool.tile([E, N], f32r, name="h0", tag="h")
                nc.scalar.activation(
                    h_cur[:],
                    dur_bc[:],
                    AF.Identity,
                    bias=binit_sb[:, 0:1],
                    scale=winit_sb[:, 0:1],
                )

                # ---------- GNN layers ----------
                for l in range(L):
                    # precedence shift: agg[i] = h[i+1], zero where i%20==19
                    agg = apool.tile([E, N], f32r, name="agg")
                    agg3 = agg[:].rearrange("p (j s) -> p j s", s=M)
                    h3 = h_cur[:].rearrange("p (j s) -> p j s", s=M)
                    nc.vector.tensor_copy(agg3[:, :, 0:M - 1], h3[:, :, 1:M])
                    nc.vector.tensor_copy(
                        agg3[:, :, M - 1], zero_r[:, 0:1].broadcast_to([E, J])
                    )
                    # token-major copies of h (for machine segment-sum)
                    htok = htokp.tile([CP, CH * E], f32r, name="htok")
                    for c in range(CH):
                        t_ps = tp_ps.tile([CP, E], f32r, name="t_ps", tag="tp")
                        nc.tensor.transpose(
                            t_ps[:],
                            h_cur[:, c * CP:(c + 1) * CP],
                            ident_r[:],
                        )
                        nc.scalar.copy(htok[:, c * E:(c + 1) * E], t_ps[:])
                    # S^T[e, m] = sum_tok h[tok, e] * P[tok, m]
                    s_ps = small_ps.tile([E, M], f32, name="s_ps", tag="sp")
                    for c in range(CH):
                        nc.tensor.matmul(
                            s_ps[:],
                            htok[:, c * E:(c + 1) * E],
                            p_all[:, c, :],
                            start=(c == 0),
                            stop=(c == CH - 1),
                        )
                    s_sb = smallsb.tile([E, M], f32r, name="s_sb")
                    nc.vector.tensor_copy(s_sb[:], s_ps[:])
                    # U = S @ Wm   [20, 128]
                    u_ps = small_ps.tile([M, E], f32, name="u_ps", tag="sp")
                    nc.tensor.matmul(u_ps[:], s_sb[:], wm_sb[l][:])
                    u_sb = smallsb.tile([M, E], f32r, name="u_sb")
                    nc.vector.tensor_copy(u_sb[:], u_ps[:])
                    # msg = relu(h (Ws-Wm) + agg Wp + P U + b)
                    msgs = []
                    for hf in range(2):
                        sl = slice(hf * HALF, (hf + 1) * HALF)
                        m_ps = mt_ps.tile([E, HALF], f32, name="m_ps", tag="mt")
                        nc.tensor.matmul(
                            m_ps[:], wsm_sb[l][:], h_cur[:, sl],
                            start=True, stop=False,
                        )
                        nc.tensor.matmul(
                            m_ps[:], wp_sb[l][:], agg[:, sl],
                            start=False, stop=False,
                        )
                        nc.tensor.matmul(
                            m_ps[:], u_sb[:], pt[:, sl], start=False, stop=True
                        )
                        msg_t = msgp.tile([E, HALF], f32r, name="msg_t")
                        nc.scalar.activation(
                            msg_t[:], m_ps[:], AF.Relu, bias=b_sb[l][:, 0:1]
                        )
                        msgs.append(msg_t)
                    # FFN: h_new = msg + relu(msg W1 + b1) W2 + b2
                    hn_pss = [
                        hn_ps.tile([E, HALF], f32, name="hn_ps0", tag="hn"),
                        hn_ps.tile([E, HALF], f32, name="hn_ps1", tag="hn"),
                    ]
                    for f in range(nf):
                        for hf in range(2):
                            tt_ps = mt_ps.tile([E, HALF], f32, name="tt_ps", tag="mt")
                            nc.tensor.matmul(
                                tt_ps[:], w1_sb[l][:, f * E:(f + 1) * E], msgs[hf][:]
                            )
                            t_sb = tpool.tile([E, HALF], f32r, name="t_sb")
                            if f % 2 == 0:
                                nc.vector.tensor_scalar(
                                    t_sb[:],
                                    tt_ps[:],
                                    b1_sb[l][:, f:f + 1],
                                    0.0,
                                    op0=OP.add,
                                    op1=OP.max,
                                )
                            else:
                                nc.scalar.activation(
                                    t_sb[:], tt_ps[:], AF.Relu,
                                    bias=b1_sb[l][:, f:f + 1],
                                )
                            nc.tensor.matmul(
                                hn_pss[hf][:],
                                w2_sb[l][:, f, :],
                                t_sb[:],
                                start=(f == 0),
                                stop=(f == nf - 1),
                            )
                    h_nxt = hpool.tile([E, N], f32r, name=f"h{l + 1}", tag="h")
                    for hf in range(2):
                        sl = slice(hf * HALF, (hf + 1) * HALF)
                        nc.vector.scalar_tensor_tensor(
                            h_nxt[:, sl],
                            hn_pss[hf][:],
                            b2_sb[l][:, 0:1],
                            msgs[hf][:],
                            op0=OP.add,
                            op1=OP.add,
                        )
                    h_cur = h_nxt

                # ---------- outputs ----------
                htok_o = htokp.tile([CP, CH * E], f32r, name="htok_o")
                for c in range(CH):
                    t_ps = tp_ps.tile([CP, E], f32r, name="t_ps", tag="tp")
                    nc.tensor.transpose(
                        t_ps[:],
                        h_cur[:, c * CP:(c + 1) * CP],
                        ident_r[:],
                    )
                    nc.scalar.copy(htok_o[:, c * E:(c + 1) * E], t_ps[:])
                je_ps = small_ps.tile([E, J], f32, name="je_ps", tag="sp")
                for c in range(CH):
                    nc.tensor.matmul(
                        je_ps[:],
                        htok_o[:, c * E:(c + 1) * E],
                        g_all[:, c, :],
                        start=(c == 0),
                        stop=(c == CH - 1),
                    )
                je_sb = smallsb.tile([E, J], f32, name="je_sb")
                nc.vector.tensor_copy(je_sb[:], je_ps[:])
                jet_ps = small_ps.tile([J, E], f32, name="jet_ps", tag="sp")
                nc.tensor.transpose(jet_ps[:], je_sb[:], ident[:])
                jet_sb = smallsb.tile([J, E], f32, name="jet_sb")
                nc.scalar.copy(jet_sb[:], jet_ps[:])
                nc.sync.dma_start(je_out[b], jet_sb[:])
                nc.sync.dma_start(
                    h_out[b].rearrange("(c p) e -> p c e", p=CP),
                    htok_o[:].bitcast(f32).rearrange("p (c e) -> p c e", e=E),
                )

    nc.compile()
    return nc


def _get_nc():
    if "nc" not in _CACHE:
        _CACHE["nc"] = _build_nc()
    return _CACHE["nc"]


def make_in_maps(proc_time, machine_order, next_op_idx, finished_jobs, params):
    proc_time = np.asarray(proc_time, dtype=np.float32).reshape(B, N)
    machine_order = np.asarray(machine_order, dtype=np.int32).reshape(B, N)
    next_op_idx = np.asarray(next_op_idx, dtype=np.int32).reshape(B, J)
    finished_jobs = np.asarray(finished_jobs).astype(np.uint8).reshape(B, J)
    wmap = {
        "W_init": np.ascontiguousarray(np.asarray(params["W_init"], np.float32)),
        "b_init": np.ascontiguousarray(np.asarray(params["b_init"], np.float32)),
    }
    for l, lp in enumerate(params["layers"]):
        wmap[f"Ws{l}"] = np.ascontiguousarray(np.asarray(lp["Ws"], np.float32))
        wmap[f"Wp{l}"] = np.ascontiguousarray(np.asarray(lp["Wp"], np.float32))
        wmap[f"Wm{l}"] = np.ascontiguousarray(np.asarray(lp["Wm"], np.float32))
        wmap[f"b{l}"] = np.ascontiguousarray(np.asarray(lp["b"], np.float32))
        wmap[f"W1{l}"] = np.ascontiguousarray(np.asarray(lp["W1"], np.float32))
        wmap[f"b1{l}"] = np.ascontiguousarray(np.asarray(lp["b1"], np.float32))
        wmap[f"W2{l}"] = np.ascontiguousarray(np.asarray(lp["W2"], np.float32))
        wmap[f"b2{l}"] = np.ascontiguousarray(np.asarray(lp["b2"], np.float32))
    in_maps = []
    for c in range(NCORES):
        sl = slice(c * BPC, (c + 1) * BPC)
        m = {
            "proc_time": np.ascontiguousarray(proc_time[sl]),
            "machine_order": np.ascontiguousarray(machine_order[sl]),
            "next_op_idx": np.ascontiguousarray(next_op_idx[sl]),
            "finished_jobs": np.ascontiguousarray(finished_jobs[sl]),
        }
        m.update(wmap)
        in_maps.append(m)
    return in_maps


def assemble(results):
    h = np.concatenate([r["h_out"] for r in results], axis=0).reshape(B, N, E)
    je = np.concatenate([r["je_out"] for r in results], axis=0).reshape(B, J, E)
    return je, h


def run_hw(in_maps, trace=False):
    from concourse.bass_utils import run_bass_kernel_spmd

    nc = _get_nc()
    return run_bass_kernel_spmd(
        nc, in_maps, core_ids=list(range(NCORES)), trace=trace
    )


def kernel(proc_time, machine_order, next_op_idx, finished_jobs, params):
    in_maps = make_in_maps(
        proc_time, machine_order, next_op_idx, finished_jobs, params
    )
    res = run_hw(in_maps, trace=False)
    return assemble(res.results)


# revision 11
# speedup vs baseline: 2.1320x; 1.1450x over previous
"""Trainium2 Bass kernel for the L2D job-shop GNN encoder.

Problem: B=64 batches, J=50 jobs x M=20 machine-ops = N=1000 nodes, E=128,
FF=512, L=3 GNN layers.  Data-parallel over 8 NeuronCores (8 batches each).

Key algebraic restructure vs. the reference:
  - adj_prec aggregation == shift h by one token (zero at job boundaries)
  - adj_mach aggregation == P @ (P^T h) - h  with P the [N,20] one-hot of
    machine ids  ->  two tiny matmuls instead of a [1000x1000] dense matmul.
    (P S) @ Wm == P @ (S @ Wm), and the "- h @ Wm" term folds into
    (Ws - Wm) as the self-weight.
Everything runs feature-major (H = h^T, [E=128 partitions, N tokens free]);
token-contractions (machine segment-sum, final gather) use 8 TensorE
transposes per batch/layer into token-major [125,128] chunks.

All large matmuls run in float32r (single-pass replicated fp32, 4x the
fp32 rate for moving dim >= 256); producers of matmul operands write
float32r so the BIR verifier's rounding requirement is met.
"""

import numpy as np

B, J, M = 64, 50, 20
N = J * M            # 1000
E, FF, L = 128, 512, 3
NCORES = 8
BPC = B // NCORES    # 8 batches per core
CH = 8               # token chunks per batch
CP = N // CH         # 125 tokens per chunk
HALF = N // 2        # 500 (psum bank = 512 fp32)
SCALE = 2048.0       # h is computed as h/SCALE on device (fp16 range safety)

_CACHE = {}


def _build_nc():
    import concourse.bass as bass  # noqa: F401
    import concourse.mybir as mybir
    import concourse.tile as tile
    from concourse import bacc
    from concourse.masks import make_identity

    dt = mybir.dt
    f32 = dt.float32
    f32r = dt.float32r  # noqa: F841
    f16 = dt.float16
    i32 = dt.int32
    u8 = dt.uint8
    AF = mybir.ActivationFunctionType
    OP = mybir.AluOpType

    nc = bacc.Bacc(
        "TRN2",
        target_bir_lowering=False,
        debug=False,
        enable_asserts=False,
        num_devices=NCORES,
    )

    proc = nc.dram_tensor("proc_time", [BPC, N], f32, kind="ExternalInput")
    mo = nc.dram_tensor("machine_order", [BPC, N], i32, kind="ExternalInput")
    nxt = nc.dram_tensor("next_op_idx", [BPC, J], i32, kind="ExternalInput")
    fin = nc.dram_tensor("finished_jobs", [BPC, J], u8, kind="ExternalInput")
    w_init = nc.dram_tensor("W_init", [1, E], f32, kind="ExternalInput")
    b_init = nc.dram_tensor("b_init", [E], f32, kind="ExternalInput")
    Ws_d, Wp_d, Wm_d, b_d, W1_d, b1_d, W2_d, b2_d = [], [], [], [], [], [], [], []
    for l in range(L):
        Ws_d.append(nc.dram_tensor(f"Ws{l}", [E, E], f32, kind="ExternalInput"))
        Wp_d.append(nc.dram_tensor(f"Wp{l}", [E, E], f32, kind="ExternalInput"))
        Wm_d.append(nc.dram_tensor(f"Wm{l}", [E, E], f32, kind="ExternalInput"))
        b_d.append(nc.dram_tensor(f"b{l}", [E], f32, kind="ExternalInput"))
        W1_d.append(nc.dram_tensor(f"W1{l}", [E, FF], f32, kind="ExternalInput"))
        b1_d.append(nc.dram_tensor(f"b1{l}", [FF], f32, kind="ExternalInput"))
        W2_d.append(nc.dram_tensor(f"W2{l}", [FF, E], f32, kind="ExternalInput"))
        b2_d.append(nc.dram_tensor(f"b2{l}", [E], f32, kind="ExternalInput"))
    h_out = nc.dram_tensor("h_out", [BPC, N, E], f16, kind="ExternalOutput")
    je_out = nc.dram_tensor("je_out", [BPC, J, E], f32, kind="ExternalOutput")

    with tile.TileContext(nc) as tc:
        with (
            tc.tile_pool(name="const", bufs=1) as const,
            tc.tile_pool(name="wpool", bufs=1) as wpool,
            tc.tile_pool(name="inp", bufs=2) as inp,
            tc.tile_pool(name="hpool", bufs=5) as hpool,
            tc.tile_pool(name="apool", bufs=2) as apool,
            tc.tile_pool(name="htokp", bufs=3) as htokp,
            tc.tile_pool(name="msgp", bufs=4) as msgp,
            tc.tile_pool(name="tpool", bufs=5) as tpool,
            tc.tile_pool(name="smallsb", bufs=3) as smallsb,
            tc.tile_pool(name="tp_ps", bufs=2, space="PSUM") as tp_ps,
            tc.tile_pool(name="small_ps", bufs=1, space="PSUM") as small_ps,
            tc.tile_pool(name="mt_ps", bufs=3, space="PSUM") as mt_ps,
            tc.tile_pool(name="hn_ps", bufs=2, space="PSUM") as hn_ps,
        ):
            # ---------------- constants ----------------
            ident = const.tile([128, 128], f32)
            make_identity(nc, ident[:])
            ones = const.tile([1, 128], f32)
            nc.gpsimd.memset(ones[:], 1.0)
            iota20i = const.tile([20, 1], i32)
            nc.gpsimd.iota(iota20i[:], pattern=[[0, 1]], base=0, channel_multiplier=1)
            iota20f = const.tile([20, 1], f32)
            nc.vector.tensor_copy(iota20f[:], iota20i[:])
            tokidi = const.tile([CP, CH], i32)
            nc.gpsimd.iota(tokidi[:], pattern=[[CP, CH]], base=0, channel_multiplier=1)
            tokidf = const.tile([CP, CH], f32)
            nc.vector.tensor_copy(tokidf[:], tokidi[:])
            iotami = const.tile([CP, CH * M], i32)
            nc.gpsimd.iota(
                iotami[:], pattern=[[0, CH], [1, M]], base=0, channel_multiplier=0
            )
            iotamf = const.tile([CP, CH * M], f32)
            nc.vector.tensor_copy(iotamf[:], iotami[:])
            iotaji = const.tile([1, J], i32)
            nc.gpsimd.iota(iotaji[:], pattern=[[M, J]], base=0, channel_multiplier=0)
            iotajf = const.tile([1, J], f32)
            nc.vector.tensor_copy(iotajf[:], iotaji[:])
            zero_f = const.tile([E, 1], f32)
            nc.gpsimd.memset(zero_f[:], 0.0)
            zero_h = const.tile([E, 1], f16)
            nc.vector.tensor_copy(zero_h[:], zero_f[:])
            ident_h = const.tile([128, 128], f16)
            nc.vector.tensor_copy(ident_h[:], ident[:])

            # ---------------- weights ----------------
            winit_sb = wpool.tile([E, 1], f32)
            nc.sync.dma_start(winit_sb[:], w_init.rearrange("o p -> p o"))
            binit_sb = wpool.tile([E, 1], f32)
            nc.sync.dma_start(binit_sb[:], b_init.rearrange("(p o) -> p o", o=1))
            wsm_sb, wp_sb, wm_sb, b_sb, w1_sb, b1_sb, w2_sb, b2_sb = (
                [], [], [], [], [], [], [], []
            )
            for l in range(L):
                ws_t = wpool.tile([E, E], f32, name=f"ws{l}")
                nc.sync.dma_start(ws_t[:], Ws_d[l][:])
                wpf_t = wpool.tile([E, E], f32, name=f"wpf{l}")
                nc.sync.dma_start(wpf_t[:], Wp_d[l][:])
                wmf_t = wpool.tile([E, E], f32, name=f"wmf{l}")
                nc.sync.dma_start(wmf_t[:], Wm_d[l][:])
                # f32r (rounded) weight copies for the PE
                wsm_t = wpool.tile([E, E], f16, name=f"wsm{l}")
                nc.vector.tensor_sub(wsm_t[:], ws_t[:], wmf_t[:])
                wp_t = wpool.tile([E, E], f16, name=f"wp{l}")
                nc.vector.tensor_copy(wp_t[:], wpf_t[:])
                wm_t = wpool.tile([E, E], f16, name=f"wm{l}")
                nc.vector.tensor_copy(wm_t[:], wmf_t[:])
                b_t = wpool.tile([E, 1], f32, name=f"b{l}")
                nc.sync.dma_start(b_t[:], b_d[l].rearrange("(p o) -> p o", o=1))
                w1f_t = wpool.tile([E, FF], f32, name=f"w1f{l}")
                nc.sync.dma_start(w1f_t[:], W1_d[l][:])
                w1_t = wpool.tile([E, FF], f16, name=f"w1{l}")
                nc.vector.tensor_copy(w1_t[:], w1f_t[:])
                b1_t = wpool.tile([E, FF // E], f32, name=f"b1{l}")
                nc.sync.dma_start(b1_t[:], b1_d[l].rearrange("(f p) -> p f", p=E))
                w2f_t = wpool.tile([E, FF // E, E], f32, name=f"w2f{l}")
                nc.sync.dma_start(w2f_t[:], W2_d[l].rearrange("(f p) e -> p f e", p=E))
                w2_t = wpool.tile([E, FF // E, E], f16, name=f"w2{l}")
                nc.vector.tensor_copy(w2_t[:], w2f_t[:])
                b2_t = wpool.tile([E, 1], f32, name=f"b2{l}")
                nc.sync.dma_start(b2_t[:], b2_d[l].rearrange("(p o) -> p o", o=1))
                wsm_sb.append(wsm_t)
                wp_sb.append(wp_t)
                wm_sb.append(wm_t)
                b_sb.append(b_t)
                w1_sb.append(w1_t)
                b1_sb.append(b1_t)
                w2_sb.append(w2_t)
                b2_sb.append(b2_t)

            nf = FF // E  # 4

            for b in range(BPC):
                # ---------- per-batch input prep ----------
                mo_bc = inp.tile([20, N], i32, name="mo_bc")
                nc.sync.dma_start(mo_bc[:], mo[b].partition_broadcast(20))
                motok_i = inp.tile([CP, CH], i32, name="motok_i")
                nc.sync.dma_start(motok_i[:], mo[b].rearrange("(c p) -> p c", p=CP))
                motok_f = inp.tile([CP, CH], f32, name="motok_f")
                nc.vector.tensor_copy(motok_f[:], motok_i[:])
                # P_all[p, c, m] = 1.0 if machine_order[token c*125+p] == m
                p_all = inp.tile([CP, CH, M], f16, name="p_all")
                nc.vector.tensor_tensor(
                    p_all[:],
                    motok_f[:][:, :, None].broadcast_to([CP, CH, M]),
                    iotamf[:].rearrange("p (c m) -> p c m", m=M),
                    op=OP.is_equal,
                )
                # PT[m, n] = 1.0 if machine_order[n] == m   [20, 1000]
                pt = inp.tile([20, N], f16, name="pt")
                nc.vector.tensor_scalar(
                    pt[:],
                    mo_bc[:],
                    iota20f[:, 0:1],
                    None,
                    op0=OP.is_equal,
                )
                # gather one-hots G[p, c, j] = 1.0 if flat[j] == c*125+p
                nxt_i = inp.tile([1, J], i32, name="nxt_i")
                nc.sync.dma_start(nxt_i[:], nxt[b][None, :])
                fin_u = inp.tile([1, J], u8, name="fin_u")
                nc.sync.dma_start(fin_u[:], fin[b][None, :])
                nxt_f = inp.tile([1, J], f32, name="nxt_f")
                nc.vector.tensor_copy(nxt_f[:], nxt_i[:])
                fin_f = inp.tile([1, J], f32, name="fin_f")
                nc.vector.tensor_copy(fin_f[:], fin_u[:])
                flat_f = inp.tile([1, J], f32, name="flat_f")
                # flat = 20*j + next + fin*(19 - next)
                nc.vector.tensor_scalar(
                    flat_f[:], nxt_f[:], -1.0, 19.0, op0=OP.mult, op1=OP.add
                )
                nc.vector.tensor_mul(flat_f[:], flat_f[:], fin_f[:])
                nc.vector.tensor_add(flat_f[:], flat_f[:], nxt_f[:])
                nc.vector.tensor_add(flat_f[:], flat_f[:], iotajf[:])
                g_all = inp.tile([CP, CH, J], f16, name="g_all")
                fb_ps = small_ps.tile([CP, J], f32, name="fb_ps", tag="sp")
                nc.tensor.matmul(fb_ps[:], ones[:1, :CP], flat_f[:1, :])
                for c in range(CH):
                    nc.vector.tensor_scalar(
                        g_all[:, c, :],
                        fb_ps[:],
                        tokidf[:, c:c + 1],
                        None,
                        op0=OP.is_equal,
                    )

                # ---------- init embed: h0 = dur * W_init + b_init ----------
                dur_bc = inp.tile([E, N], f32, name="dur_bc")
                nc.sync.dma_start(dur_bc[:], proc[b].partition_broadcast(E))
                h_cur = hpool.tile([E, N], f16, name="h0", tag="h")
                nc.scalar.activation(
                    h_cur[:],
                    dur_bc[:],
                    AF.Identity,
                    bias=binit_sb[:, 0:1],
                    scale=winit_sb[:, 0:1],
                )

                # ---------- GNN layers ----------
                for l in range(L):
                    # precedence shift: agg[i] = h[i+1], zero where i%20==19
                    agg = apool.tile([E, N], f16, name="agg")
                    agg3 = agg[:].rearrange("p (j s) -> p j s", s=M)
                    h3 = h_cur[:].rearrange("p (j s) -> p j s", s=M)
                    nc.vector.tensor_copy(agg3[:, :, 0:M - 1], h3[:, :, 1:M])
                    nc.vector.tensor_copy(
                        agg3[:, :, M - 1], zero_h[:, 0:1].broadcast_to([E, J])
                    )
                    # token-major copies of h (for machine segment-sum)
                    htok = htokp.tile([CP, CH * E], f16, name="htok")
                    for c in range(CH):
                        t_ps = tp_ps.tile([CP, E], f16, name="t_ps", tag="tp")
                        nc.tensor.transpose(
                            t_ps[:],
                            h_cur[:, c * CP:(c + 1) * CP],
                            ident_h[:],
                        )
                        nc.scalar.copy(htok[:, c * E:(c + 1) * E], t_ps[:])
                    # S^T[e, m] = sum_tok h[tok, e] * P[tok, m]
                    s_ps = small_ps.tile([E, M], f32, name="s_ps", tag="sp")
                    for c in range(CH):
                        nc.tensor.matmul(
                            s_ps[:],
                            htok[:, c * E:(c + 1) * E],
                            p_all[:, c, :],
                            start=(c == 0),
                            stop=(c == CH - 1),
                        )
                    s_sb = smallsb.tile([E, M], f16, name="s_sb")
                    nc.vector.tensor_copy(s_sb[:], s_ps[:])
                    # U = S @ Wm   [20, 128]
                    u_ps = small_ps.tile([M, E], f32, name="u_ps", tag="sp")
                    nc.tensor.matmul(u_ps[:], s_sb[:], wm_sb[l][:])
                    u_sb = smallsb.tile([M, E], f16, name="u_sb")
                    nc.vector.tensor_copy(u_sb[:], u_ps[:])
                    # msg = relu(h (Ws-Wm) + agg Wp + P U + b)
                    msgs = []
                    for hf in range(2):
                        sl = slice(hf * HALF, (hf + 1) * HALF)
                        m_ps = mt_ps.tile([E, HALF], f32, name="m_ps", tag="mt")
                        nc.tensor.matmul(
                            m_ps[:], wsm_sb[l][:], h_cur[:, sl],
                            start=True, stop=False,
                        )
                        nc.tensor.matmul(
                            m_ps[:], wp_sb[l][:], agg[:, sl],
                            start=False, stop=False,
                        )
                        nc.tensor.matmul(
                            m_ps[:], u_sb[:], pt[:, sl], start=False, stop=True
                        )
                        msg_t = msgp.tile([E, HALF], f16, name="msg_t")
                        nc.scalar.activation(
                            msg_t[:], m_ps[:], AF.Relu, bias=b_sb[l][:, 0:1]
                        )
                        msgs.append(msg_t)
                    # FFN: h_new = msg + relu(msg W1 + b1) W2 + b2
                    hn_pss = [
                        hn_ps.tile([E, HALF], f32, name="hn_ps0", tag="hn"),
                        hn_ps.tile([E, HALF], f32, name="hn_ps1", tag="hn"),
                    ]
                    for f in range(nf):
                        for hf in range(2):
                            tt_ps = mt_ps.tile([E, HALF], f32, name="tt_ps", tag="mt")
                            nc.tensor.matmul(
                                tt_ps[:], w1_sb[l][:, f * E:(f + 1) * E], msgs[hf][:]
                            )
                            t_sb = tpool.tile([E, HALF], f16, name="t_sb")
                            if f % 2 == 0:
                                nc.vector.tensor_scalar(
                                    t_sb[:],
                                    tt_ps[:],
                                    b1_sb[l][:, f:f + 1],
                                    0.0,
                                    op0=OP.add,
                                    op1=OP.max,
                                )
                            else:
                                nc.scalar.activation(
                                    t_sb[:], tt_ps[:], AF.Relu,
                                    bias=b1_sb[l][:, f:f + 1],
                                )
                            nc.tensor.matmul(
                                hn_pss[hf][:],
                                w2_sb[l][:, f, :],
                                t_sb[:],
                                start=(f == 0),
                                stop=(f == nf - 1),
                            )
                    h_nxt = hpool.tile([E, N], f16, name=f"h{l + 1}", tag="h")
                    for hf in range(2):
                        sl = slice(hf * HALF, (hf + 1) * HALF)
                        nc.vector.scalar_tensor_tensor(
                            h_nxt[:, sl],
                            hn_pss[hf][:],
                            b2_sb[l][:, 0:1],
                            msgs[hf][:],
                            op0=OP.add,
                            op1=OP.add,
                        )
                    h_cur = h_nxt

                # ---------- outputs ----------
                htok_o = htokp.tile([CP, CH * E], f16, name="htok_o")
                for c in range(CH):
                    t_ps = tp_ps.tile([CP, E], f16, name="t_ps", tag="tp")
                    nc.tensor.transpose(
                        t_ps[:],
                        h_cur[:, c * CP:(c + 1) * CP],
                        ident_h[:],
                    )
                    nc.scalar.copy(htok_o[:, c * E:(c + 1) * E], t_ps[:])
                je_ps = small_ps.tile([E, J], f32, name="je_ps", tag="sp")
                for c in range(CH):
                    nc.tensor.matmul(
                        je_ps[:],
                        htok_o[:, c * E:(c + 1) * E],
                        g_all[:, c, :],
                        start=(c == 0),
                        stop=(c == CH - 1),
                    )
                je_sb = smallsb.tile([E, J], f32, name="je_sb")
                nc.vector.tensor_scalar_mul(je_sb[:], je_ps[:], SCALE)
                jet_ps = small_ps.tile([J, E], f32, name="jet_ps", tag="sp")
                nc.tensor.transpose(jet_ps[:], je_sb[:], ident[:])
                jet_sb = smallsb.tile([J, E], f32, name="jet_sb")
                nc.scalar.copy(jet_sb[:], jet_ps[:])
                nc.sync.dma_start(je_out[b], jet_sb[:])
                nc.sync.dma_start(
                    h_out[b].rearrange("(c p) e -> p c e", p=CP),
                    htok_o[:].rearrange("p (c e) -> p c e", e=E),
                )

    nc.compile()
    return nc


def _get_nc():
    if "nc" not in _CACHE:
        _CACHE["nc"] = _build_nc()
    return _CACHE["nc"]


def make_in_maps(proc_time, machine_order, next_op_idx, finished_jobs, params):
    proc_time = np.asarray(proc_time, dtype=np.float32).reshape(B, N)
    machine_order = np.asarray(machine_order, dtype=np.int32).reshape(B, N)
    next_op_idx = np.asarray(next_op_idx, dtype=np.int32).reshape(B, J)
    finished_jobs = np.asarray(finished_jobs).astype(np.uint8).reshape(B, J)
    inv = np.float32(1.0 / SCALE)
    wmap = {
        "W_init": np.ascontiguousarray(np.asarray(params["W_init"], np.float32) * inv),
        "b_init": np.ascontiguousarray(np.asarray(params["b_init"], np.float32) * inv),
    }
    for l, lp in enumerate(params["layers"]):
        wmap[f"Ws{l}"] = np.ascontiguousarray(np.asarray(lp["Ws"], np.float32))
        wmap[f"Wp{l}"] = np.ascontiguousarray(np.asarray(lp["Wp"], np.float32))
        wmap[f"Wm{l}"] = np.ascontiguousarray(np.asarray(lp["Wm"], np.float32))
        wmap[f"b{l}"] = np.ascontiguousarray(np.asarray(lp["b"], np.float32) * inv)
        wmap[f"W1{l}"] = np.ascontiguousarray(np.asarray(lp["W1"], np.float32))
        wmap[f"b1{l}"] = np.ascontiguousarray(np.asarray(lp["b1"], np.float32) * inv)
        wmap[f"W2{l}"] = np.ascontiguousarray(np.asarray(lp["W2"], np.float32))
        wmap[f"b2{l}"] = np.ascontiguousarray(np.asarray(lp["b2"], np.float32) * inv)
    in_maps = []
    for c in range(NCORES):
        sl = slice(c * BPC, (c + 1) * BPC)
        m = {
            "proc_time": np.ascontiguousarray(proc_time[sl]),
            "machine_order": np.ascontiguousarray(machine_order[sl]),
            "next_op_idx": np.ascontiguousarray(next_op_idx[sl]),
            "finished_jobs": np.ascontiguousarray(finished_jobs[sl]),
        }
        m.update(wmap)
        in_maps.append(m)
    return in_maps


def assemble(results):
    h16 = np.concatenate([r["h_out"] for r in results], axis=0).reshape(B, N, E)
    h = h16.astype(np.float32) * np.float32(SCALE)
    je = np.concatenate([r["je_out"] for r in results], axis=0).reshape(B, J, E)
    return je, h


def run_hw(in_maps, trace=False):
    from concourse.bass_utils import run_bass_kernel_spmd

    nc = _get_nc()
    return run_bass_kernel_spmd(
        nc, in_maps, core_ids=list(range(NCORES)), trace=trace
    )


def kernel(proc_time, machine_order, next_op_idx, finished_jobs, params):
    in_maps = make_in_maps(
        proc_time, machine_order, next_op_idx, finished_jobs, params
    )
    res = run_hw(in_maps, trace=False)
    return assemble(res.results)


# revision 12
# speedup vs baseline: 2.8856x; 1.3535x over previous
"""Trainium2 Bass kernel for the L2D job-shop GNN encoder.

Problem: B=64 batches, J=50 jobs x M=20 machine-ops = N=1000 nodes, E=128,
FF=512, L=3 GNN layers.  Data-parallel over 8 NeuronCores (8 batches each).

Key algebraic restructure vs. the reference:
  - adj_prec aggregation == shift h by one token (zero at job boundaries)
  - adj_mach aggregation == P @ (P^T h) - h  with P the [N,20] one-hot of
    machine ids  ->  two tiny matmuls instead of a [1000x1000] dense matmul.
    (P S) @ Wm == P @ (S @ Wm), and the "- h @ Wm" term folds into
    (Ws - Wm) as the self-weight.
Everything runs feature-major (H = h^T, [E=128 partitions, N tokens free]);
token-contractions (machine segment-sum, final gather) use 8 TensorE
transposes per batch/layer into token-major [125,128] chunks.

All large matmuls run in float32r (single-pass replicated fp32, 4x the
fp32 rate for moving dim >= 256); producers of matmul operands write
float32r so the BIR verifier's rounding requirement is met.
"""

import numpy as np

B, J, M = 64, 50, 20
N = J * M            # 1000
E, FF, L = 128, 512, 3
NCORES = 8
BPC = B // NCORES    # 8 batches per core
CH = 8               # token chunks per batch
CP = N // CH         # 125 tokens per chunk
HALF = N // 2        # 500 (psum bank = 512 fp32)
SCALE = 2048.0       # h is computed as h/SCALE on device (fp16 range safety)

_CACHE = {}


def _build_nc():
    import concourse.bass as bass  # noqa: F401
    import concourse.mybir as mybir
    import concourse.tile as tile
    from concourse import bacc
    from concourse.masks import make_identity

    dt = mybir.dt
    f32 = dt.float32
    f32r = dt.float32r  # noqa: F841
    f16 = dt.float16
    i32 = dt.int32
    u8 = dt.uint8
    AF = mybir.ActivationFunctionType
    OP = mybir.AluOpType

    nc = bacc.Bacc(
        "TRN2",
        target_bir_lowering=False,
        debug=False,
        enable_asserts=False,
        num_devices=NCORES,
    )

    proc = nc.dram_tensor("proc_time", [BPC, N], f32, kind="ExternalInput")
    mo = nc.dram_tensor("machine_order", [BPC, N], i32, kind="ExternalInput")
    nxt = nc.dram_tensor("next_op_idx", [BPC, J], i32, kind="ExternalInput")
    fin = nc.dram_tensor("finished_jobs", [BPC, J], u8, kind="ExternalInput")
    w_init = nc.dram_tensor("W_init", [1, E], f32, kind="ExternalInput")
    b_init = nc.dram_tensor("b_init", [E], f32, kind="ExternalInput")
    Ws_d, Wp_d, Wm_d, b_d, W1_d, b1_d, W2_d, b2_d = [], [], [], [], [], [], [], []
    for l in range(L):
        Ws_d.append(nc.dram_tensor(f"Ws{l}", [E, E], f32, kind="ExternalInput"))
        Wp_d.append(nc.dram_tensor(f"Wp{l}", [E, E], f32, kind="ExternalInput"))
        Wm_d.append(nc.dram_tensor(f"Wm{l}", [E, E], f32, kind="ExternalInput"))
        b_d.append(nc.dram_tensor(f"b{l}", [E], f32, kind="ExternalInput"))
        W1_d.append(nc.dram_tensor(f"W1{l}", [E, FF], f32, kind="ExternalInput"))
        b1_d.append(nc.dram_tensor(f"b1{l}", [FF], f32, kind="ExternalInput"))
        W2_d.append(nc.dram_tensor(f"W2{l}", [FF, E], f32, kind="ExternalInput"))
        b2_d.append(nc.dram_tensor(f"b2{l}", [E], f32, kind="ExternalInput"))
    h_out = nc.dram_tensor("h_out", [BPC, N, E], f16, kind="ExternalOutput")
    je_out = nc.dram_tensor("je_out", [BPC, J, E], f32, kind="ExternalOutput")

    with tile.TileContext(nc) as tc:
        with (
            tc.tile_pool(name="const", bufs=1) as const,
            tc.tile_pool(name="wpool", bufs=1) as wpool,
            tc.tile_pool(name="inp", bufs=2) as inp,
            tc.tile_pool(name="persist", bufs=BPC + 1) as persist,
            tc.tile_pool(name="hpool", bufs=BPC + 2) as hpool,
            tc.tile_pool(name="apool", bufs=3) as apool,
            tc.tile_pool(name="htokp", bufs=3) as htokp,
            tc.tile_pool(name="msgp", bufs=4) as msgp,
            tc.tile_pool(name="tpool", bufs=5) as tpool,
            tc.tile_pool(name="smallsb", bufs=3) as smallsb,
            tc.tile_pool(name="tp_ps", bufs=2, space="PSUM") as tp_ps,
            tc.tile_pool(name="small_ps", bufs=2, space="PSUM") as small_ps,
            tc.tile_pool(name="mt_ps", bufs=2, space="PSUM") as mt_ps,
            tc.tile_pool(name="hn_ps", bufs=2, space="PSUM") as hn_ps,
        ):
            # ---------------- constants ----------------
            ident = const.tile([128, 128], f32)
            make_identity(nc, ident[:])
            ones = const.tile([1, 128], f32)
            nc.gpsimd.memset(ones[:], 1.0)
            iota20i = const.tile([20, 1], i32)
            nc.gpsimd.iota(iota20i[:], pattern=[[0, 1]], base=0, channel_multiplier=1)
            iota20f = const.tile([20, 1], f32)
            nc.vector.tensor_copy(iota20f[:], iota20i[:])
            tokidi = const.tile([CP, CH], i32)
            nc.gpsimd.iota(tokidi[:], pattern=[[CP, CH]], base=0, channel_multiplier=1)
            tokidf = const.tile([CP, CH], f32)
            nc.vector.tensor_copy(tokidf[:], tokidi[:])
            iotami = const.tile([CP, CH * M], i32)
            nc.gpsimd.iota(
                iotami[:], pattern=[[0, CH], [1, M]], base=0, channel_multiplier=0
            )
            iotamf = const.tile([CP, CH * M], f32)
            nc.vector.tensor_copy(iotamf[:], iotami[:])
            iotaji = const.tile([1, J], i32)
            nc.gpsimd.iota(iotaji[:], pattern=[[M, J]], base=0, channel_multiplier=0)
            iotajf = const.tile([1, J], f32)
            nc.vector.tensor_copy(iotajf[:], iotaji[:])
            zero_f = const.tile([E, 1], f32)
            nc.gpsimd.memset(zero_f[:], 0.0)
            zero_h = const.tile([E, 1], f16)
            nc.vector.tensor_copy(zero_h[:], zero_f[:])
            ident_h = const.tile([128, 128], f16)
            nc.vector.tensor_copy(ident_h[:], ident[:])

            # ---------------- weights ----------------
            winit_sb = wpool.tile([E, 1], f32)
            nc.sync.dma_start(winit_sb[:], w_init.rearrange("o p -> p o"))
            binit_sb = wpool.tile([E, 1], f32)
            nc.sync.dma_start(binit_sb[:], b_init.rearrange("(p o) -> p o", o=1))
            wsm_sb, wp_sb, wm_sb, b_sb, w1_sb, b1_sb, w2_sb, b2_sb = (
                [], [], [], [], [], [], [], []
            )
            for l in range(L):
                ws_t = wpool.tile([E, E], f32, name=f"ws{l}")
                nc.sync.dma_start(ws_t[:], Ws_d[l][:])
                wpf_t = wpool.tile([E, E], f32, name=f"wpf{l}")
                nc.sync.dma_start(wpf_t[:], Wp_d[l][:])
                wmf_t = wpool.tile([E, E], f32, name=f"wmf{l}")
                nc.sync.dma_start(wmf_t[:], Wm_d[l][:])
                # f32r (rounded) weight copies for the PE
                wsm_t = wpool.tile([E, E], f16, name=f"wsm{l}")
                nc.vector.tensor_sub(wsm_t[:], ws_t[:], wmf_t[:])
                wp_t = wpool.tile([E, E], f16, name=f"wp{l}")
                nc.vector.tensor_copy(wp_t[:], wpf_t[:])
                wm_t = wpool.tile([E, E], f16, name=f"wm{l}")
                nc.vector.tensor_copy(wm_t[:], wmf_t[:])
                b_t = wpool.tile([E, 1], f32, name=f"b{l}")
                nc.sync.dma_start(b_t[:], b_d[l].rearrange("(p o) -> p o", o=1))
                w1f_t = wpool.tile([E, FF], f32, name=f"w1f{l}")
                nc.sync.dma_start(w1f_t[:], W1_d[l][:])
                w1_t = wpool.tile([E, FF], f16, name=f"w1{l}")
                nc.vector.tensor_copy(w1_t[:], w1f_t[:])
                b1_t = wpool.tile([E, FF // E], f32, name=f"b1{l}")
                nc.sync.dma_start(b1_t[:], b1_d[l].rearrange("(f p) -> p f", p=E))
                w2f_t = wpool.tile([E, FF // E, E], f32, name=f"w2f{l}")
                nc.sync.dma_start(w2f_t[:], W2_d[l].rearrange("(f p) e -> p f e", p=E))
                w2_t = wpool.tile([E, FF // E, E], f16, name=f"w2{l}")
                nc.vector.tensor_copy(w2_t[:], w2f_t[:])
                b2_t = wpool.tile([E, 1], f32, name=f"b2{l}")
                nc.sync.dma_start(b2_t[:], b2_d[l].rearrange("(p o) -> p o", o=1))
                wsm_sb.append(wsm_t)
                wp_sb.append(wp_t)
                wm_sb.append(wm_t)
                b_sb.append(b_t)
                w1_sb.append(w1_t)
                b1_sb.append(b1_t)
                w2_sb.append(w2_t)
                b2_sb.append(b2_t)

            nf = FF // E  # 4

            # ---------- prep: one-hots + h0 for ALL batches ----------
            pts, p_alls, g_alls, h_curs = [], [], [], []
            for b in range(BPC):
                mo_bc = inp.tile([20, N], i32, name="mo_bc")
                nc.sync.dma_start(mo_bc[:], mo[b].partition_broadcast(20))
                motok_i = inp.tile([CP, CH], i32, name="motok_i")
                nc.sync.dma_start(motok_i[:], mo[b].rearrange("(c p) -> p c", p=CP))
                motok_f = inp.tile([CP, CH], f32, name="motok_f")
                nc.vector.tensor_copy(motok_f[:], motok_i[:])
                p_all = persist.tile([CP, CH, M], f16, name="p_all")
                nc.vector.tensor_tensor(
                    p_all[:],
                    motok_f[:][:, :, None].broadcast_to([CP, CH, M]),
                    iotamf[:].rearrange("p (c m) -> p c m", m=M),
                    op=OP.is_equal,
                )
                pt = persist.tile([20, N], f16, name="pt")
                nc.vector.tensor_scalar(
                    pt[:], mo_bc[:], iota20f[:, 0:1], None, op0=OP.is_equal
                )
                nxt_i = inp.tile([1, J], i32, name="nxt_i")
                nc.sync.dma_start(nxt_i[:], nxt[b][None, :])
                fin_u = inp.tile([1, J], u8, name="fin_u")
                nc.sync.dma_start(fin_u[:], fin[b][None, :])
                nxt_f = inp.tile([1, J], f32, name="nxt_f")
                nc.vector.tensor_copy(nxt_f[:], nxt_i[:])
                fin_f = inp.tile([1, J], f32, name="fin_f")
                nc.vector.tensor_copy(fin_f[:], fin_u[:])
                flat_f = inp.tile([1, J], f32, name="flat_f")
                nc.vector.tensor_scalar(
                    flat_f[:], nxt_f[:], -1.0, 19.0, op0=OP.mult, op1=OP.add
                )
                nc.vector.tensor_mul(flat_f[:], flat_f[:], fin_f[:])
                nc.vector.tensor_add(flat_f[:], flat_f[:], nxt_f[:])
                nc.vector.tensor_add(flat_f[:], flat_f[:], iotajf[:])
                g_all = persist.tile([CP, CH, J], f16, name="g_all")
                fb_ps = small_ps.tile([CP, J], f32, name="fb_ps", tag="sp")
                nc.tensor.matmul(fb_ps[:], ones[:1, :CP], flat_f[:1, :])
                for c in range(CH):
                    nc.vector.tensor_scalar(
                        g_all[:, c, :], fb_ps[:], tokidf[:, c:c + 1], None,
                        op0=OP.is_equal,
                    )
                dur_bc = inp.tile([E, N], f32, name="dur_bc")
                nc.sync.dma_start(dur_bc[:], proc[b].partition_broadcast(E))
                h_cur = hpool.tile([E, N], f16, name="h0", tag="h")
                nc.scalar.activation(
                    h_cur[:], dur_bc[:], AF.Identity,
                    bias=binit_sb[:, 0:1], scale=winit_sb[:, 0:1],
                )
                pts.append(pt)
                p_alls.append(p_all)
                g_alls.append(g_all)
                h_curs.append(h_cur)

            # ---------- software-pipelined layers ----------
            # "stall" = shift + transposes + segment-sum S + U for one (b, l):
            # sparse PE work that would starve the array (HAM re-throttle) if
            # emitted as a block, so it is interleaved into the previous
            # batch's dense msg/FFN matmul stream via thunks.
            def emit_stall(b, l):
                h_in = h_curs[b]
                st = {}
                thunks = []

                def sh():
                    agg = apool.tile([E, N], f16, name="agg", tag="agg")
                    agg3 = agg[:].rearrange("p (j s) -> p j s", s=M)
                    h3 = h_in[:].rearrange("p (j s) -> p j s", s=M)
                    nc.vector.tensor_copy(agg3[:, :, 0:M - 1], h3[:, :, 1:M])
                    nc.vector.tensor_copy(
                        agg3[:, :, M - 1], zero_h[:, 0:1].broadcast_to([E, J])
                    )
                    st["agg"] = agg
                    st["htok"] = htokp.tile([CP, CH * E], f16, name="htok",
                                            tag="htok")

                thunks.append(sh)

                def mk_tc(c):
                    def tcop():
                        t_ps = tp_ps.tile([CP, E], f16, name="t_ps", tag="tp")
                        nc.tensor.transpose(
                            t_ps[:], h_in[:, c * CP:(c + 1) * CP], ident_h[:]
                        )
                        dst = st["htok"][:, c * E:(c + 1) * E]
                        if c % 2 == 0:
                            nc.scalar.copy(dst, t_ps[:])
                        else:
                            nc.vector.tensor_copy(dst, t_ps[:])
                    return tcop

                for c in range(CH):
                    thunks.append(mk_tc(c))

                def mk_s(c0):
                    def smm():
                        if c0 == 0:
                            st["s_ps"] = small_ps.tile([E, M], f32, name="s_ps",
                                                       tag="sp")
                        for c in range(c0, c0 + 4):
                            nc.tensor.matmul(
                                st["s_ps"][:],
                                st["htok"][:, c * E:(c + 1) * E],
                                p_alls[b][:, c, :],
                                start=(c == 0),
                                stop=(c == CH - 1),
                            )
                    return smm

                thunks.append(mk_s(0))
                thunks.append(mk_s(4))

                def ufin():
                    s_sb = smallsb.tile([E, M], f16, name="s_sb")
                    nc.vector.tensor_copy(s_sb[:], st["s_ps"][:])
                    u_ps = small_ps.tile([M, E], f32, name="u_ps", tag="sp")
                    nc.tensor.matmul(u_ps[:], s_sb[:], wm_sb[l][:])
                    u_sb = smallsb.tile([M, E], f16, name="u_sb")
                    nc.vector.tensor_copy(u_sb[:], u_ps[:])
                    st["u_sb"] = u_sb

                thunks.append(ufin)
                return thunks, st

            def emit_out(b):
                h_in = h_curs[b]
                st = {}
                thunks = []

                def start():
                    st["htok_o"] = htokp.tile([CP, CH * E], f16, name="htok_o",
                                              tag="htok")

                thunks.append(start)

                def mk_tc(c):
                    def tcop():
                        t_ps = tp_ps.tile([CP, E], f16, name="t_ps", tag="tp")
                        nc.tensor.transpose(
                            t_ps[:], h_in[:, c * CP:(c + 1) * CP], ident_h[:]
                        )
                        dst = st["htok_o"][:, c * E:(c + 1) * E]
                        if c % 2 == 0:
                            nc.scalar.copy(dst, t_ps[:])
                        else:
                            nc.vector.tensor_copy(dst, t_ps[:])
                    return tcop

                for c in range(CH):
                    thunks.append(mk_tc(c))

                def mk_g(c0):
                    def gmm():
                        if c0 == 0:
                            st["je_ps"] = small_ps.tile([E, J], f32,
                                                        name="je_ps", tag="sp")
                        for c in range(c0, c0 + 4):
                            nc.tensor.matmul(
                                st["je_ps"][:],
                                st["htok_o"][:, c * E:(c + 1) * E],
                                g_alls[b][:, c, :],
                                start=(c == 0),
                                stop=(c == CH - 1),
                            )
                    return gmm

                thunks.append(mk_g(0))
                thunks.append(mk_g(4))

                def fin():
                    je_sb = smallsb.tile([E, J], f32, name="je_sb")
                    nc.vector.tensor_scalar_mul(je_sb[:], st["je_ps"][:], SCALE)
                    jet_ps = small_ps.tile([J, E], f32, name="jet_ps", tag="sp")
                    nc.tensor.transpose(jet_ps[:], je_sb[:], ident[:])
                    jet_sb = smallsb.tile([J, E], f32, name="jet_sb")
                    nc.scalar.copy(jet_sb[:], jet_ps[:])
                    nc.sync.dma_start(je_out[b], jet_sb[:])
                    nc.sync.dma_start(
                        h_out[b].rearrange("(c p) e -> p c e", p=CP),
                        st["htok_o"][:].rearrange("p (c e) -> p c e", e=E),
                    )

                thunks.append(fin)
                return thunks

            def dense(b, l, st, pend):
                def fill(k):
                    for _ in range(k):
                        if pend:
                            pend.pop(0)()

                h_in = h_curs[b]
                agg = st["agg"]
                msgs = []
                for hf in range(2):
                    sl = slice(hf * HALF, (hf + 1) * HALF)
                    m_ps = mt_ps.tile([E, HALF], f32, name="m_ps", tag="mt")
                    nc.tensor.matmul(
                        m_ps[:], wsm_sb[l][:], h_in[:, sl], start=True, stop=False
                    )
                    fill(1)
                    nc.tensor.matmul(
                        m_ps[:], wp_sb[l][:], agg[:, sl], start=False, stop=False
                    )
                    fill(1)
                    nc.tensor.matmul(
                        m_ps[:], st["u_sb"][:], pts[b][:, sl],
                        start=False, stop=True,
                    )
                    msg_t = msgp.tile([E, HALF], f16, name="msg_t")
                    nc.scalar.activation(
                        msg_t[:], m_ps[:], AF.Relu, bias=b_sb[l][:, 0:1]
                    )
                    msgs.append(msg_t)
                    fill(1)
                hn_pss = [
                    hn_ps.tile([E, HALF], f32, name="hn_ps0", tag="hn"),
                    hn_ps.tile([E, HALF], f32, name="hn_ps1", tag="hn"),
                ]
                for f in range(nf):
                    for hf in range(2):
                        tt_ps = mt_ps.tile([E, HALF], f32, name="tt_ps", tag="mt")
                        nc.tensor.matmul(
                            tt_ps[:], w1_sb[l][:, f * E:(f + 1) * E], msgs[hf][:]
                        )
                        t_sb = tpool.tile([E, HALF], f16, name="t_sb")
                        if f % 2 == 0:
                            nc.vector.tensor_scalar(
                                t_sb[:], tt_ps[:], b1_sb[l][:, f:f + 1], 0.0,
                                op0=OP.add, op1=OP.max,
                            )
                        else:
                            nc.scalar.activation(
                                t_sb[:], tt_ps[:], AF.Relu,
                                bias=b1_sb[l][:, f:f + 1],
                            )
                        nc.tensor.matmul(
                            hn_pss[hf][:], w2_sb[l][:, f, :], t_sb[:],
                            start=(f == 0), stop=(f == nf - 1),
                        )
                    fill(2)
                h_nxt = hpool.tile([E, N], f16, name=f"h{l + 1}", tag="h")
                for hf in range(2):
                    sl = slice(hf * HALF, (hf + 1) * HALF)
                    nc.vector.scalar_tensor_tensor(
                        h_nxt[:, sl], hn_pss[hf][:], b2_sb[l][:, 0:1],
                        msgs[hf][:], op0=OP.add, op1=OP.add,
                    )
                h_curs[b] = h_nxt
                fill(len(pend))

            thunks, st = emit_stall(0, 0)
            for t in thunks:
                t()
            stall_sts = {(0, 0): st}
            for l in range(L):
                for b in range(BPC):
                    st = stall_sts.pop((b, l))
                    if l < L - 1:
                        nb, nl = (b + 1, l) if b + 1 < BPC else (0, l + 1)
                        pend, pst = emit_stall(nb, nl)
                        stall_sts[(nb, nl)] = pst
                    elif b + 1 < BPC:
                        # last layer: stall(b+1, L-1) still needed
                        pend, pst = emit_stall(b + 1, l)
                        stall_sts[(b + 1, l)] = pend_extra = pst
                        # also interleave output of previous batch
                        if b >= 1:
                            pend = pend + emit_out(b - 1)
                    else:
                        pend = emit_out(b - 1) if b >= 1 else []
                    dense(b, l, st, pend)
            for t in emit_out(BPC - 1):
                t()

    nc.compile()
    return nc


def _get_nc():
    if "nc" not in _CACHE:
        _CACHE["nc"] = _build_nc()
    return _CACHE["nc"]


def make_in_maps(proc_time, machine_order, next_op_idx, finished_jobs, params):
    proc_time = np.asarray(proc_time, dtype=np.float32).reshape(B, N)
    machine_order = np.asarray(machine_order, dtype=np.int32).reshape(B, N)
    next_op_idx = np.asarray(next_op_idx, dtype=np.int32).reshape(B, J)
    finished_jobs = np.asarray(finished_jobs).astype(np.uint8).reshape(B, J)
    inv = np.float32(1.0 / SCALE)
    wmap = {
        "W_init": np.ascontiguousarray(np.asarray(params["W_init"], np.float32) * inv),
        "b_init": np.ascontiguousarray(np.asarray(params["b_init"], np.float32) * inv),
    }
    for l, lp in enumerate(params["layers"]):
        wmap[f"Ws{l}"] = np.ascontiguousarray(np.asarray(lp["Ws"], np.float32))
        wmap[f"Wp{l}"] = np.ascontiguousarray(np.asarray(lp["Wp"], np.float32))
        wmap[f"Wm{l}"] = np.ascontiguousarray(np.asarray(lp["Wm"], np.float32))
        wmap[f"b{l}"] = np.ascontiguousarray(np.asarray(lp["b"], np.float32) * inv)
        wmap[f"W1{l}"] = np.ascontiguousarray(np.asarray(lp["W1"], np.float32))
        wmap[f"b1{l}"] = np.ascontiguousarray(np.asarray(lp["b1"], np.float32) * inv)
        wmap[f"W2{l}"] = np.ascontiguousarray(np.asarray(lp["W2"], np.float32))
        wmap[f"b2{l}"] = np.ascontiguousarray(np.asarray(lp["b2"], np.float32) * inv)
    in_maps = []
    for c in range(NCORES):
        sl = slice(c * BPC, (c + 1) * BPC)
        m = {
            "proc_time": np.ascontiguousarray(proc_time[sl]),
            "machine_order": np.ascontiguousarray(machine_order[sl]),
            "next_op_idx": np.ascontiguousarray(next_op_idx[sl]),
            "finished_jobs": np.ascontiguousarray(finished_jobs[sl]),
        }
        m.update(wmap)
        in_maps.append(m)
    return in_maps


def assemble(results):
    h16 = np.concatenate([r["h_out"] for r in results], axis=0).reshape(B, N, E)
    h = h16.astype(np.float32) * np.float32(SCALE)
    je = np.concatenate([r["je_out"] for r in results], axis=0).reshape(B, J, E)
    return je, h


def run_hw(in_maps, trace=False):
    from concourse.bass_utils import run_bass_kernel_spmd

    nc = _get_nc()
    return run_bass_kernel_spmd(
        nc, in_maps, core_ids=list(range(NCORES)), trace=trace
    )


def kernel(proc_time, machine_order, next_op_idx, finished_jobs, params):
    in_maps = make_in_maps(
        proc_time, machine_order, next_op_idx, finished_jobs, params
    )
    res = run_hw(in_maps, trace=False)
    return assemble(res.results)
